# revision 43
# baseline (speedup 1.0000x reference)
"""BERT(2-layer) + CRF NLL loss kernel for Trainium2, data-parallel over batch on 8 cores.

fp8 (e4m3) DoubleRow matmuls for all linear layers (halves PSUM accumulation
passes: K=256 per instruction), bf16 pre-LN accumulators, fp8 residual stream
(scale 1, weights x64), DVE-only rsqrt for LN (bit-trick seed + 1 Newton step,
768-scale folded into the Newton constants -- no Ln/Exp act-table loads on the
LN critical path), QK bias folded into the PSUM-drain tensor_scalar.

Layout per core (2 examples, 1024 token-slots), all feature-major:
  - h8   fp8 [128, 2, 1024] x3: post-LN activations, pair i = feature k-tile 2p+i.
  - q8/k8 fp8 [96, 2, 1024] x4: tile a holds heads 3a..3a+2 at partition bases
    0/32/64 (PE cannot read base 96), pair i = dh 32i+r, via a host-side
    wqkv column permutation; scores contract DH=64 as [32,2] DoubleRow.
  - v8 fp8 [128, 2, 768] x4 token-groups (token (2u+i)*128+p); softmax
    denominator via a separate ones[128,2,32] DoubleRow matmul (DoubleRow
    stationary must be >=32 columns).
  - ctx8 fp8 x3 natural feature pairs; xres bf16 x6: pre-LN residual accumulators.
  - weights fp8 x64 in DoubleRow pair layout [128, 2, out]; Wo/W2/V biases via
    small bf16 rank-1 matmuls (64*b rows x ones) accumulated into PSUM; W1
    bias via the gelu activation bias; QK bias via per-partition tensor_scalar.
  - FF runs in 256-token chunks so W1/gelu/W2 pipeline inside 8 PSUM banks.
  - CRF identical to v1 (log-domain associative scan over 9x9 matrices).

Known hw caveats found on the way: DoubleRow is 1 cycle/row on TRN2 (cost
model says 0.5) -- the win is 2x K per instruction, not faster rows; the
chip power-throttles the PE to ~50% duty for ~2/3 of this kernel (dense
PE-only streams do not trip it); tensor_tensor_reduce crashes the device;
engine ops need partition bases in {0,32,64,96}.
"""

import sys

sys.path.insert(0, "/opt/trn_rl_repo")

import numpy as np
import ml_dtypes

import concourse.bass as bass
import concourse.tile as tile
from concourse import bacc, mybir
from concourse.bass import AP
from concourse.bass_utils import run_bass_kernel_spmd
from concourse.masks import make_identity

F32 = mybir.dt.float32
BF16 = mybir.dt.bfloat16
FP8 = mybir.dt.float8e4
I32 = mybir.dt.int32
AF = mybir.ActivationFunctionType
ALU = mybir.AluOpType
AX = mybir.AxisListType
DR = mybir.MatmulPerfMode.DoubleRow

P = 128
B, S, D, L, H, T, V = 16, 512, 768, 2, 12, 9, 30522
DH = D // H          # 64
FF = 4 * D           # 3072
NCORES = 8
BL = B // NCORES     # 2 examples per core
NTOK = BL * S        # 1024
KD = D // P          # 6 k-tiles over D
KP = KD // 2         # 3 k-pair tiles
FFP = FF // 256      # 12 ff-pair tiles
NT = NTOK // 512     # 2 n-chunks of 512 tokens
TT = NTOK // P       # 8 token-tiles
EPS = 1e-12
NEG = -1000.0        # effective -inf for log-domain CRF
G = 8                # CRF scan steps per chunk
CCH = 64             # chunks per example
NSTEP = 510          # scan steps (S'-1 where S'=511)
EMROWS = NTOK + 16   # em output padded so chunk loads never go OOB
WS = 64.0            # fp8 weight scale
IWS = 1.0 / WS
ES = 16.0            # exp tile scale
LN16 = float(np.log(ES))
LN768 = float(np.log(768.0))
EPS_S = 768.0 * 768.0 * EPS

def _bitrev(n, bits):
    r = 0
    for _ in range(bits):
        r = (r << 1) | (n & 1)
        n >>= 1
    return r

_BITREV7 = np.array([_bitrev(p, 7) for p in range(128)], dtype=np.int64)

# q/k output-feature permutation: tile a holds heads 3a..3a+2 on partitions
# 32j+r (j=head%3, base 0/32/64 only -- PE cannot read from base 96), pair i
# selects dh 32i+r. Permuted column a*192 + i*96 + j*32 + r <- head 3a+j,
# dh i*32+r.
_PQK = np.empty(D, dtype=np.int64)
for _a in range(4):
    for _i in range(2):
        for _j in range(3):
            for _r in range(32):
                _PQK[_a * 192 + _i * 96 + _j * 32 + _r] = \
                    (3 * _a + _j) * 64 + _i * 32 + _r


# ----------------------------------------------------------------------------
# device program
# ----------------------------------------------------------------------------

def build_program():
    nc = bacc.Bacc("TRN2", target_bir_lowering=False, debug=False)

    def din(name, shape, dt):
        return nc.dram_tensor(name, shape, dt, kind="ExternalInput").ap()

    def dout(name, shape, dt):
        return nc.dram_tensor(name, shape, dt, kind="ExternalOutput").ap()

    t = dict(
        tok=din("tok", [NTOK, 1], I32),
        wemb=din("wemb", [V, D], F32),
        pos=din("pos", [S, D], F32),
        wqkv8=din("wqkv8", [L, KP, P, 2, 3 * D], FP8),
        wo8=din("wo8", [L, KP, P, 2, D], FP8),
        w18=din("w18", [L, KP, P, 2, FF], FP8),
        w28=din("w28", [L, FFP, P, 2, D], FP8),
        wtag8=din("wtag8", [KP, P, 2, 32], FP8),
        qkbT=din("qkbT", [L, 96, 16], F32),      # 64*b, permuted, per-tile cols
        vbB=din("vbB", [L, 1, D], BF16),         # 64*b_v
        boB=din("boB", [L, 1, D], BF16),
        b2B=din("b2B", [L, 1, D], BF16),
        b1T=din("b1T", [L, P, FF // P], F32),
        lnsT=din("lnsT", [2 * L + 1, P, KD], F32),
        lnbT=din("lnbT", [2 * L + 1, P, KD], F32),
        btag=din("btag", [T, 1], F32),
        transB=din("transB", [P, 81], F32),
        ilogB=din("ilogB", [P, 81], F32),
        maskB=din("maskB", [P, G], F32),
        start2=din("start2", [BL, T], F32),
        end2=din("end2", [BL, T], F32),
        selT=din("selT", [T, NTOK], F32),
        permC=din("permC", [P, 1], I32),
        emS=nc.dram_tensor("emS", [P, G * T], F32, kind="Internal").ap(),
        em=dout("em", [EMROWS, T], F32),
        numdot=dout("numdot", [T, 1], F32),
        logz=dout("logz", [BL, 1], F32),
    )

    with tile.TileContext(nc) as tc:
        _emit(nc, tc, t)
    nc.compile()
    return nc


def _emit(nc, tc, t):
    from contextlib import ExitStack

    with ExitStack() as ctx:
        const = ctx.enter_context(tc.tile_pool(name="const", bufs=1))
        hpool = ctx.enter_context(tc.tile_pool(name="h", bufs=1))
        wpool = ctx.enter_context(tc.tile_pool(name="w", bufs=1))

        ident = const.tile([P, P], F32, name="ident", tag="ident")
        make_identity(nc, ident[:])
        identb = const.tile([P, P], BF16, name="identb", tag="identb")
        make_identity(nc, identb[:])
        ones_row = const.tile([1, 512], BF16, name="ones_row", tag="ones_row")
        nc.vector.memset(ones_row[:], 1.0)
        ones1b = const.tile([1, P], BF16, name="ones1b", tag="ones1b")
        nc.vector.memset(ones1b[:], 1.0)
        c768row = const.tile([1, P], BF16, name="c768row", tag="c768row")
        nc.vector.memset(c768row[:], 1.0 / 768.0)
        ones128b = const.tile([P, 1], BF16, name="ones128b", tag="ones128b")
        nc.vector.memset(ones128b[:], 1.0)
        epsS = const.tile([1, 1], F32, name="epsS", tag="epsS")
        nc.vector.memset(epsS[:], EPS_S)
        ln768t = const.tile([1, 1], F32, name="ln768t", tag="ln768t")
        nc.vector.memset(ln768t[:], LN768)
        ln16t = const.tile([P, 1], F32, name="ln16t", tag="ln16t")
        nc.vector.memset(ln16t[:], LN16)
        cbias = dict(epsS=epsS, ln768t=ln768t, ln16t=ln16t)  # + ones8 below

        # persistent activation tiles
        h8 = [hpool.tile([P, 2, NTOK], FP8, name=f"h8_{p}", tag=f"h8_{p}")
              for p in range(KP)]
        q8 = [hpool.tile([96, 2, NTOK], FP8, name=f"q8_{a}", tag=f"q8_{a}")
              for a in range(4)]
        k8 = [hpool.tile([96, 2, NTOK], FP8, name=f"k8_{a}", tag=f"k8_{a}")
              for a in range(4)]
        v8 = [hpool.tile([P, 2, H * DH], FP8, name=f"v8_{u}", tag=f"v8_{u}")
              for u in range(4)]
        ones8 = const.tile([P, 2, 32], FP8, name="ones8", tag="ones8")
        nc.vector.memset(ones8[:], 1.0)
        cbias["ones8"] = ones8
        ctx8 = [hpool.tile([P, 2, NTOK], FP8, name=f"ctx8_{p}", tag=f"ctx8_{p}")
                for p in range(KP)]
        xres = [hpool.tile([P, NTOK], BF16, name=f"xres{k}", tag=f"xres{k}")
                for k in range(KD)]

        # LN scale/bias param tiles ([P, site, k])
        lns = const.tile([P, 2 * L + 1, KD], F32, name="lns", tag="lns")
        nc.sync.dma_start(lns[:], t["lnsT"].rearrange("a p k -> p a k"))
        lnb = const.tile([P, 2 * L + 1, KD], F32, name="lnb", tag="lnb")
        nc.sync.dma_start(lnb[:], t["lnbT"].rearrange("a p k -> p a k"))

        # ------------------------------------------------------------------
        # embedding: gather + pos, cast bf16, transpose to xres, then LN
        # ------------------------------------------------------------------
        with tc.tile_pool(name="emb", bufs=2) as emb, \
             tc.tile_pool(name="embps", bufs=3, space="PSUM") as embps, \
             tc.tile_pool(name="posp", bufs=1) as posp:
            pos_t = []
            for q in range(S // P):
                pt = posp.tile([P, D], F32, name=f"pos{q}", tag=f"pos{q}")
                nc.sync.dma_start(pt[:], t["pos"][q * P:(q + 1) * P, :])
                pos_t.append(pt)
            for n in range(NT):
                gbfs = []
                for q in range(4):
                    tt_i = n * 4 + q
                    idx = emb.tile([P, 1], I32, name="idx", tag="idx")
                    nc.sync.dma_start(idx[:], t["tok"][tt_i * P:(tt_i + 1) * P, :])
                    g32 = emb.tile([P, D], F32, name="g32", tag="g32")
                    nc.gpsimd.indirect_dma_start(
                        out=g32[:], out_offset=None, in_=t["wemb"][:],
                        in_offset=bass.IndirectOffsetOnAxis(ap=idx[:, :1], axis=0),
                    )
                    gbf = emb.tile([P, D], BF16, name=f"gbf{q}", tag=f"gbf{q}")
                    nc.vector.tensor_add(gbf[:], g32[:], pos_t[tt_i % 4][:])
                    gbfs.append(gbf)
                for k in range(KD):
                    pb = embps.tile([P, 4, P], BF16, name="pb", tag="pb",
                                    space="PSUM")
                    for q in range(4):
                        nc.tensor.transpose(
                            pb[:, q, :], gbfs[q][:, k * P:(k + 1) * P], identb[:])
                    nc.vector.tensor_copy(
                        xres[k][:, n * 512:(n + 1) * 512],
                        pb[:].rearrange("p q c -> p (q c)"))
        for n in range(NT):
            _ln_apply(nc, tc, n, xres, h8, lns[:, 0, :], lnb[:, 0, :],
                      ones128b, ones1b, c768row, cbias)

        # ------------------------------------------------------------------
        # weights to SBUF (fp8)
        # ------------------------------------------------------------------
        wq8 = [[wpool.tile([P, 2, 3 * D], FP8, name=f"wq{l}_{p}", tag=f"wq{l}_{p}")
                for p in range(KP)] for l in range(L)]
        wo8 = [[wpool.tile([P, 2, D], FP8, name=f"wo{l}_{p}", tag=f"wo{l}_{p}")
                for p in range(KP)] for l in range(L)]
        w18 = [[wpool.tile([P, 2, FF], FP8, name=f"w1{l}_{p}", tag=f"w1{l}_{p}")
                for p in range(KP)] for l in range(L)]
        w28 = [[wpool.tile([P, 2, D], FP8, name=f"w2{l}_{f}", tag=f"w2{l}_{f}")
                for f in range(FFP)] for l in range(L)]
        for l in range(L):
            for p in range(KP):
                nc.sync.dma_start(wq8[l][p][:], t["wqkv8"][l, p])
                nc.sync.dma_start(wo8[l][p][:], t["wo8"][l, p])
                nc.sync.dma_start(w18[l][p][:], t["w18"][l, p])
            for f in range(FFP):
                nc.sync.dma_start(w28[l][f][:], t["w28"][l, f])
        qkb = const.tile([96, L, 16], F32, name="qkb", tag="qkb")
        nc.sync.dma_start(qkb[:], t["qkbT"].rearrange("l p m -> p l m"))
        vbb = const.tile([1, L, D], BF16, name="vbb", tag="vbb")
        nc.sync.dma_start(vbb[:], t["vbB"].rearrange("l o d -> o l d"))
        bob = const.tile([1, L, D], BF16, name="bob", tag="bob")
        nc.sync.dma_start(bob[:], t["boB"].rearrange("l o d -> o l d"))
        b2b = const.tile([1, L, D], BF16, name="b2b", tag="b2b")
        nc.sync.dma_start(b2b[:], t["b2B"].rearrange("l o d -> o l d"))
        b1t = const.tile([P, L, FF // P], F32, name="b1t", tag="b1t")
        nc.sync.dma_start(b1t[:], t["b1T"].rearrange("l p k -> p l k"))

        # ------------------------------------------------------------------
        # encoder layers
        # ------------------------------------------------------------------
        for l in range(L):
            _layer(nc, tc, t, l, h8, q8, k8, v8, ctx8, xres,
                   wq8[l], wo8[l], w18[l], w28[l],
                   qkb, vbb[:, l, :], bob[:, l, :], b2b[:, l, :],
                   b1t[:, l, :],
                   lns, lnb, ones_row, ones1b, c768row, ones128b, cbias)

        # ------------------------------------------------------------------
        # emissions: em = wtag.T @ h + btag  (feature-major [9, NTOK])
        # ------------------------------------------------------------------
        with tc.tile_pool(name="emp", bufs=1) as emp, \
             tc.tile_pool(name="emps", bufs=2, space="PSUM") as emps:
            wtg = [emp.tile([P, 2, 32], FP8, name=f"wtg{p}", tag=f"wtg{p}")
                   for p in range(KP)]
            for p in range(KP):
                nc.sync.dma_start(wtg[p][:], t["wtag8"][p])
            btg = emp.tile([T, 1], F32, name="btg", tag="btg")
            nc.sync.dma_start(btg[:], t["btag"][:])
            em_sb = emp.tile([T, NTOK], F32, name="em_sb", tag="em_sb")
            for n in range(NT):
                ps = emps.tile([32, 512], F32, name="emmm", tag="emmm",
                               space="PSUM")
                for p in range(KP):
                    nc.tensor.matmul(
                        ps[:], lhsT=wtg[p][:],
                        rhs=h8[p][:, :, n * 512:(n + 1) * 512],
                        start=(p == 0), stop=(p == KP - 1), perf_mode=DR)
                nc.scalar.activation(
                    em_sb[:, n * 512:(n + 1) * 512], ps[:T, :], AF.Identity,
                    bias=btg[:, :1], scale=IWS)
            # numerator dot: sum(em * selT) fused multiply-reduce
            sel = emp.tile([T, NTOK], F32, name="sel", tag="sel")
            nc.sync.dma_start(sel[:], t["selT"][:])
            prod = emp.tile([T, NTOK], F32, name="prod", tag="prod")
            nc.vector.tensor_mul(prod[:], em_sb[:], sel[:])
            nd = emp.tile([T, 1], F32, name="nd", tag="nd")
            nc.vector.reduce_sum(out=nd[:], in_=prod[:], axis=AX.X)
            nc.sync.dma_start(t["numdot"][:], nd[:])
            # token-major em to DRAM (+ zero pad rows)
            zpad = emp.tile([16, T], F32, name="zpad", tag="zpad")
            nc.vector.memset(zpad[:], 0.0)
            nc.sync.dma_start(t["em"][NTOK:EMROWS, :], zpad[:])
            for tt_i in range(TT):
                tp = emps.tile([P, T], F32, name="emtp", tag="emtp", space="PSUM")
                nc.tensor.transpose(
                    tp[:], em_sb[:, tt_i * P:(tt_i + 1) * P], ident[:T, :T])
                emtm = emp.tile([P, T], F32, name="emtm", tag="emtm", bufs=3)
                nc.vector.tensor_copy(emtm[:], tp[:])
                nc.sync.dma_start(t["em"][tt_i * P:(tt_i + 1) * P, :], emtm[:])

        # ------------------------------------------------------------------
        # CRF forward pass (log-domain associative scan)
        # ------------------------------------------------------------------
        _crf(nc, tc, t)


def _ln_apply(nc, tc, n, xres, h8, sT, bT, ones128b, ones1b, c768row, cbias):
    """Feature-major LN of xres (bf16) chunk n -> h8 (fp8).

    rstd computed as exp(-0.5 ln(768*sq - mu^2) + ln 768).
    """
    sl = slice(n * 512, (n + 1) * 512)
    with tc.tile_pool(name="lnp", bufs=1) as lnp, \
         tc.tile_pool(name="lnps", bufs=1, space="PSUM") as lnps:
        mu_ps = lnps.tile([1, 512], F32, name="mu", tag="mu", space="PSUM")
        sq_ps = lnps.tile([1, 512], F32, name="sq", tag="sq", space="PSUM")
        xsq = [lnp.tile([P, 512], BF16, name=f"xsq{k}", tag=f"xsq{k}", bufs=1)
               for k in range(KD)]
        for k in range(KD):
            nc.vector.tensor_mul(xsq[k][:], xres[k][:, sl], xres[k][:, sl])
        for k in range(KD):
            nc.tensor.matmul(mu_ps[:], lhsT=ones128b[:], rhs=xres[k][:, sl],
                             start=(k == 0), stop=(k == KD - 1))
        for k in range(KD):
            nc.tensor.matmul(sq_ps[:], lhsT=ones128b[:], rhs=xsq[k][:],
                             start=(k == 0), stop=(k == KD - 1))
        musq = lnp.tile([1, 512], F32, name="musq", tag="musq")
        nc.scalar.square(musq[:], mu_ps[:])
        svar = lnp.tile([1, 512], F32, name="svar", tag="svar")
        nc.vector.scalar_tensor_tensor(
            out=svar[:], in0=sq_ps[:], scalar=768.0, in1=musq[:],
            op0=ALU.mult, op1=ALU.subtract)
        # rstd = 768*rsqrt(svar) via bit-trick seed + one Newton step, all
        # on DVE (no Ln/Exp -> no act-table loads on the LN critical path)
        q1 = lnp.tile([1, 512], I32, name="q1", tag="q1")
        nc.vector.tensor_scalar(
            out=q1[:], in0=svar[:].bitcast(I32), scalar1=1, scalar2=None,
            op0=ALU.logical_shift_right)
        q2 = lnp.tile([1, 512], I32, name="q2", tag="q2")
        nc.vector.tensor_scalar(
            out=q2[:], in0=q1[:], scalar1=-1, scalar2=None,
            op0=ALU.bitwise_xor)
        y0 = lnp.tile([1, 512], F32, name="y0", tag="y0")
        nc.vector.tensor_scalar(
            out=y0[:].bitcast(I32), in0=q2[:], scalar1=0x5F3759E0,
            scalar2=None, op0=ALU.add)
        n1 = lnp.tile([1, 512], F32, name="n1", tag="n1")
        nc.vector.tensor_mul(n1[:], y0[:], y0[:])
        n2 = lnp.tile([1, 512], F32, name="n2", tag="n2")
        nc.vector.tensor_mul(n2[:], n1[:], svar[:])
        n3 = lnp.tile([1, 512], F32, name="n3", tag="n3")
        nc.vector.tensor_scalar(
            out=n3[:], in0=n2[:], scalar1=-384.0, scalar2=1152.0,
            op0=ALU.mult, op1=ALU.add)
        rs = lnp.tile([1, 512], BF16, name="rs", tag="rs")
        nc.vector.tensor_mul(rs[:], y0[:], n3[:])
        murs = lnp.tile([1, 512], BF16, name="murs", tag="murs")
        nc.vector.tensor_mul(murs[:], mu_ps[:], rs[:])
        rsB_ps = lnps.tile([P, 512], F32, name="rsB", tag="rsB", space="PSUM")
        nc.tensor.matmul(rsB_ps[:], lhsT=ones1b[:], rhs=rs[:],
                         start=True, stop=True)
        m2_ps = lnps.tile([P, 512], F32, name="m2B", tag="m2B", space="PSUM")
        nc.tensor.matmul(m2_ps[:], lhsT=c768row[:], rhs=murs[:],
                         start=True, stop=True)
        rsB = lnp.tile([P, 512], BF16, name="rsBs", tag="rsBs")
        nc.vector.tensor_copy(rsB[:], rsB_ps[:])
        m2B = lnp.tile([P, 512], BF16, name="m2Bs", tag="m2Bs")
        nc.vector.tensor_copy(m2B[:], m2_ps[:])
        for k in range(KD):
            tm = lnp.tile([P, 512], BF16, name="tm", tag="tm", bufs=3)
            nc.vector.tensor_mul(tm[:], xres[k][:, sl], rsB[:])
            ts = lnp.tile([P, 512], BF16, name="ts", tag="ts", bufs=3)
            nc.vector.tensor_sub(ts[:], tm[:], m2B[:])
            dst = h8[k // 2][:, k % 2, sl]
            nc.scalar.activation(dst, ts[:], AF.Identity,
                                 bias=bT[:, k:k + 1], scale=sT[:, k:k + 1])


def _layer(nc, tc, t, l, h8, q8, k8, v8, ctx8, xres,
           wq8, wo8, w18, w28, qkb, vbb, bob, b2b, b1t,
           lns, lnb, ones_row, ones1b, c768row, ones128b, cbias):
    # --------------- QKV: q8/k8 feature-major perm, v8 token-major ---------
    with tc.tile_pool(name="qkps", bufs=4, space="PSUM") as qkps:
        for n in range(NT):
            nsl = slice(n * 512, (n + 1) * 512)
            for m in range(16):       # 8 Q tiles then 8 K tiles (96-part, perm)
                csl = slice(m * 96, (m + 1) * 96)
                ps = qkps.tile([96, 512], F32, name="ps", tag="ps", space="PSUM")
                for p in range(KP):
                    nc.tensor.matmul(
                        ps[:], lhsT=wq8[p][:, :, csl],
                        rhs=h8[p][:, :, nsl],
                        start=(p == 0), stop=(p == KP - 1), perf_mode=DR,
                        skip_group_check=True)
                dstq = q8 if m < 8 else k8
                mm = m % 8
                nc.vector.tensor_scalar(
                    out=dstq[mm // 2][:, mm % 2, nsl], in0=ps[:],
                    scalar1=qkb[:, l, m:m + 1], scalar2=IWS,
                    op0=ALU.add, op1=ALU.mult)
        for tt_i in range(TT):        # V token-major
            for nv in range(2):
                vsl = slice(2 * D + nv * 384, 2 * D + (nv + 1) * 384)
                ps = qkps.tile([P, 384], F32, name="psv", tag="psv", space="PSUM")
                nc.tensor.matmul(ps[:], lhsT=ones1b[:],
                                 rhs=vbb[:, nv * 384:(nv + 1) * 384],
                                 start=True, stop=False, skip_group_check=True)
                for p in range(KP):
                    nc.tensor.matmul(
                        ps[:], lhsT=h8[p][:, :, tt_i * P:(tt_i + 1) * P],
                        rhs=wq8[p][:, :, vsl],
                        start=False, stop=(p == KP - 1), perf_mode=DR,
                        skip_group_check=True)
                nc.vector.tensor_scalar_mul(
                    v8[tt_i // 2][:, tt_i % 2, nv * 384:(nv + 1) * 384],
                    ps[:], IWS)

    # --------------- attention ----------------------------------------
    with tc.tile_pool(name="att", bufs=1) as att, \
         tc.tile_pool(name="scps", bufs=2, space="PSUM") as scps, \
         tc.tile_pool(name="ctxps", bufs=2, space="PSUM") as ctxps, \
         tc.tile_pool(name="denps", bufs=1, space="PSUM") as denps, \
         tc.tile_pool(name="invps", bufs=1, space="PSUM") as invps:
        for b in range(BL):
            bsl = slice(b * S, (b + 1) * S)
            for hp in range(H // 2):
                cps = []
                ivs = []
                for hh in range(2):
                    h = hp * 2 + hh
                    a, j = h // 3, h % 3
                    psl = slice(32 * j, 32 * j + 32)
                    cp = ctxps.tile([DH, 512], F32, name="ctx", tag="ctx",
                                    space="PSUM")
                    den = denps.tile([32, 512], F32, name="den", tag="den",
                                     space="PSUM")
                    for u in range(2):
                        sc = scps.tile([P, 2, 512], F32, name="sc", tag="sc",
                                       space="PSUM")
                        for i in range(2):
                            kt = 2 * u + i
                            nc.tensor.matmul(
                                sc[:, i, :],
                                lhsT=k8[a][psl, :,
                                           b * S + kt * P:b * S + (kt + 1) * P],
                                rhs=q8[a][psl, :, bsl],
                                start=True, stop=True, perf_mode=DR)
                        et = att.tile([P, 2, 512], FP8, name="et", tag="et",
                                      bufs=4)
                        nc.scalar.activation(
                            et[:].rearrange("p i q -> p (i q)"),
                            sc[:].rearrange("p i q -> p (i q)"),
                            AF.Exp, scale=0.125, bias=cbias["ln16t"][:, :1])
                        nc.tensor.matmul(
                            cp[:],
                            lhsT=v8[2 * b + u][:, :, h * DH:(h + 1) * DH],
                            rhs=et[:], start=(u == 0), stop=(u == 1),
                            perf_mode=DR, skip_group_check=True)
                        nc.tensor.matmul(
                            den[:], lhsT=cbias["ones8"][:], rhs=et[:],
                            start=(u == 0), stop=(u == 1),
                            perf_mode=DR, skip_group_check=True)
                    iv = att.tile([1, 512], F32, name="iv", tag="iv", bufs=4)
                    nc.vector.reciprocal_approx_fast(iv[:], den[:1, :])
                    ivb = att.tile([1, 512], BF16, name="ivb", tag="ivb",
                                   bufs=4)
                    nc.vector.tensor_copy(ivb[:], iv[:])
                    cps.append(cp)
                    ivs.append(ivb)
                ivB = invps.tile([P, 512], F32, name="ivB", tag="ivB",
                                 space="PSUM")
                nc.tensor.matmul(ivB[:DH, :], lhsT=ones1b[:, :DH],
                                 rhs=ivs[0][:], start=True, stop=True)
                nc.tensor.matmul(ivB[DH:, :], lhsT=ones1b[:, :DH],
                                 rhs=ivs[1][:], start=True, stop=True)
                ivS = att.tile([P, 512], BF16, name="ivS", tag="ivS", bufs=2)
                nc.scalar.copy(ivS[:], ivB[:])
                for hh in range(2):
                    h = hp * 2 + hh
                    po = (h % 2) * DH
                    nc.vector.tensor_mul(
                        ctx8[h // 4][po:po + DH, (h // 2) % 2, bsl],
                        cps[hh][:, :], ivS[po:po + DH, :])

    # --------------- Wo + residual + LN1 -------------------------------
    with tc.tile_pool(name="wops", bufs=3, space="PSUM") as wops:
        for n in range(NT):
            nsl = slice(n * 512, (n + 1) * 512)
            for m in range(KD):
                ps = wops.tile([P, 512], F32, name="ps", tag="ps", space="PSUM")
                nc.tensor.matmul(ps[:], lhsT=bob[:, m * P:(m + 1) * P],
                                 rhs=ones_row[:], start=True, stop=False,
                                 skip_group_check=True)
                for p in range(KP):
                    nc.tensor.matmul(
                        ps[:], lhsT=wo8[p][:, :, m * P:(m + 1) * P],
                        rhs=ctx8[p][:, :, nsl],
                        start=False, stop=(p == KP - 1), perf_mode=DR,
                        skip_group_check=True)
                nc.vector.scalar_tensor_tensor(
                    out=xres[m][:, nsl], in0=ps[:], scalar=IWS,
                    in1=h8[m // 2][:, m % 2, nsl], op0=ALU.mult, op1=ALU.add)
            _ln_apply(nc, tc, n, xres, h8, lns[:, 2 * l + 1, :],
                      lnb[:, 2 * l + 1, :], ones128b, ones1b, c768row, cbias)

    # --------------- FF (256-token chunks, pipelined in PSUM) -----------
    with tc.tile_pool(name="ffac", bufs=1, space="PSUM") as ffac, \
         tc.tile_pool(name="ffps", bufs=1, space="PSUM") as ffps, \
         tc.tile_pool(name="ffg", bufs=3) as ffg:
        acc_t = ffac.tile([P, KD, 256], F32, name="acc", tag="acc",
                          space="PSUM")
        acc = [acc_t[:, m, :] for m in range(KD)]
        for c in range(4):
            csl = slice(c * 256, (c + 1) * 256)
            for m in range(KD):
                nc.tensor.matmul(acc[m],
                                 lhsT=b2b[:, m * P:(m + 1) * P],
                                 rhs=ones_row[:, :256], start=True, stop=False,
                                 skip_group_check=True)
            for f in range(FFP):
                psg = ffps.tile([P, 2, 256], F32, name="psg", tag="psg",
                                space="PSUM")
                for i in range(2):
                    for p in range(KP):
                        nc.tensor.matmul(
                            psg[:, i, :],
                            lhsT=w18[p][:, :, (2 * f + i) * P:(2 * f + i + 1) * P],
                            rhs=h8[p][:, :, csl],
                            start=(p == 0), stop=(p == KP - 1), perf_mode=DR)
                gl = ffg.tile([P, 2, 256], FP8, name="gl", tag="gl")
                for i in range(2):
                    nc.scalar.activation(
                        gl[:, i, :], psg[:, i, :], AF.Gelu,
                        bias=b1t[:, 2 * f + i:2 * f + i + 1], scale=IWS)
                for m in range(KD):
                    nc.tensor.matmul(
                        acc[m], lhsT=w28[f][:, :, m * P:(m + 1) * P],
                        rhs=gl[:], start=False, stop=(f == FFP - 1),
                        perf_mode=DR, skip_group_check=True)
            for m in range(KD):
                nc.vector.scalar_tensor_tensor(
                    out=xres[m][:, csl], in0=acc[m], scalar=IWS,
                    in1=h8[m // 2][:, m % 2, csl], op0=ALU.mult, op1=ALU.add)
            if c % 2 == 1:
                _ln_apply(nc, tc, c // 2, xres, h8, lns[:, 2 * l + 2, :],
                          lnb[:, 2 * l + 2, :], ones128b, ones1b, c768row,
                          cbias)


def _crf_combine_lin(nc, out_ap, a_ap, b_ap, spool, npart, npair):
    """Linear-domain combine: out[i,j] = sum_k A[i,k]*B[k,j] (no Act ops)."""
    s = spool.tile([P, 4, 729], F32, name="cS", tag="cS")
    sv4 = s[:npart, :npair, :].rearrange("p q (x k) -> p q x k", k=T)
    for q in range(npair):
        avq = a_ap[:, q].rearrange("p (i k) -> p i k", i=T)
        avq = avq.unsqueeze(2).broadcast_to([npart, T, T, T])    # p i j k
        bvq = b_ap[:, q].rearrange("p (k j) -> p k j", k=T)
        bvq = bvq.unsqueeze(1).broadcast_to([npart, T, T, T])    # p i k j
        bvq = bvq.transpose([0, 1, 3, 2])                        # p i j k
        svq = s[:npart, q, :].rearrange("p (i j k) -> p i j k", i=T, j=T)
        nc.vector.tensor_tensor(out=svq, in0=avq, in1=bvq, op=ALU.mult)
    nc.vector.reduce_sum(out=out_ap, in_=sv4, axis=AX.X)


def _crf_combine(nc, out_ap, a_ap, b_ap, spool, npart, npair, stabilize=True):
    """out = A 'logmatmul' B over pairs: out[i,j] = lse_k(A[i,k]+B[k,j])."""
    s = spool.tile([P, 4, 729], F32, name="cS", tag="cS")
    sv4 = s[:npart, :npair, :].rearrange("p q (x k) -> p q x k", k=T)
    sv3 = s[:npart, :npair, :]
    for q in range(npair):
        avq = a_ap[:, q].rearrange("p (i k) -> p i k", i=T)
        avq = avq.unsqueeze(2).broadcast_to([npart, T, T, T])    # p i j k
        bvq = b_ap[:, q].rearrange("p (k j) -> p k j", k=T)
        bvq = bvq.unsqueeze(1).broadcast_to([npart, T, T, T])    # p i k j
        bvq = bvq.transpose([0, 1, 3, 2])                        # p i j k
        svq = s[:npart, q, :].rearrange("p (i j k) -> p i j k", i=T, j=T)
        nc.vector.tensor_tensor(out=svq, in0=avq, in1=bvq, op=ALU.add)
    sm = spool.tile([P, 4, 81], F32, name="cR", tag="cR")
    sm3 = sm[:npart, :npair, :]
    if stabilize:
        mx = spool.tile([P, 4, 81], F32, name="cM", tag="cM")
        mx3 = mx[:npart, :npair, :]
        nc.vector.reduce_max(out=mx3, in_=sv4, axis=AX.X)
        mxv = mx3.unsqueeze(3).broadcast_to([npart, npair, 81, T])
        nc.vector.tensor_tensor(out=sv4, in0=sv4, in1=mxv, op=ALU.subtract)
        nc.scalar.activation(sv3, sv3, AF.Exp)
        nc.vector.reduce_sum(out=sm3, in_=sv4, axis=AX.X)
        nc.scalar.activation(sm3, sm3, AF.Ln)
        nc.vector.tensor_tensor(out=out_ap, in0=sm3, in1=mx3, op=ALU.add)
    else:
        nc.scalar.activation(sv3, sv3, AF.Exp)
        nc.vector.reduce_sum(out=sm3, in_=sv4, axis=AX.X)
        nc.scalar.activation(out_ap, sm3, AF.Ln)


def _crf(nc, tc, t):
    """Log-domain associative scan. Partitions 0..63 = example0 chunks,
    64..127 = example1 chunks; each chunk = G=8 consecutive scan steps."""
    with tc.tile_pool(name="crf", bufs=1) as crf, \
         tc.tile_pool(name="crfs", bufs=1) as crfs:
        transB = crf.tile([P, 81], F32, name="transB", tag="transB")
        nc.sync.dma_start(transB[:], t["transB"][:])
        ilogB = crf.tile([P, 81], F32, name="ilogB", tag="ilogB")
        nc.sync.dma_start(ilogB[:], t["ilogB"][:])
        maskB = crf.tile([P, G], F32, name="maskB", tag="maskB")
        nc.sync.dma_start(maskB[:], t["maskB"][:])

        shifted = AP(t["em"].tensor, 2 * T, [[G * T, P], [1, G * T]])
        nc.sync.dma_start(t["emS"][:], shifted)
        permt = crf.tile([P, 1], I32, name="permt", tag="permt")
        nc.sync.dma_start(permt[:], t["permC"][:])
        e2 = crf.tile([P, G * T], F32, name="e2", tag="e2")
        nc.gpsimd.indirect_dma_start(
            out=e2[:], out_offset=None, in_=t["emS"][:],
            in_offset=bass.IndirectOffsetOnAxis(ap=permt[:, :1], axis=0),
        )

        # M[c, g, i, j] = ilog + mask*(trans + e - ilog)
        m0 = crf.tile([P, G, 81], F32, name="m0", tag="m0")
        mv = m0[:].rearrange("p g (i j) -> p g i j", i=T)
        e2v = e2[:].rearrange("p (g j) -> p g j", g=G)
        e2v = e2v.unsqueeze(2).broadcast_to([P, G, T, T])
        trv = transB[:].rearrange("p (i j) -> p i j", i=T)
        trv = trv.unsqueeze(1).broadcast_to([P, G, T, T])
        nc.vector.tensor_tensor(out=mv, in0=trv, in1=e2v, op=ALU.add)
        ilv = ilogB[:].rearrange("p (i j) -> p i j", i=T)
        ilv = ilv.unsqueeze(1).broadcast_to([P, G, T, T])
        nc.vector.tensor_tensor(out=mv, in0=mv, in1=ilv, op=ALU.subtract)
        mkv = maskB[:].unsqueeze(2).unsqueeze(3).broadcast_to([P, G, T, T])
        nc.vector.tensor_tensor(out=mv, in0=mv, in1=mkv, op=ALU.mult)
        nc.vector.tensor_tensor(out=mv, in0=mv, in1=ilv, op=ALU.add)

        # in-chunk combines 8 -> 4 -> 2 -> 1 in LINEAR domain (one upfront
        # exp, multiply+reduce only; path sums bounded so fp32 never
        # overflows: chunk products <= ~2e16, one cross level <= ~5e33)
        mlin = crf.tile([P, G, 81], F32, name="mlin", tag="mlin")
        nc.scalar.activation(mlin[:].rearrange("p g x -> p (g x)"),
                             m0[:].rearrange("p g x -> p (g x)"), AF.Exp)
        cur = mlin
        width = G
        lvl = 0
        while width > 1:
            width //= 2
            nxt = crf.tile([P, width, 81], F32, name=f"ml{lvl}", tag=f"ml{lvl}")
            pairs = cur[:].rearrange("p a x -> p a x")
            av = pairs[:, 0:2 * width:2, :]
            bv = pairs[:, 1:2 * width:2, :]
            _crf_combine_lin(nc, nxt[:], av, bv, crfs, P, width)
            cur = nxt
            lvl += 1

        # first cross-chunk level (128 -> 64) still linear, then to log
        cur_ap = cur[:].rearrange("p a x -> p (a x)")   # [128, 81]
        bL = crf.tile([P, 81], F32, name="tbL", tag="tbL")
        nc.sync.dma_start(bL[:64, :], cur_ap[64:128])
        nxtL = crf.tile([P, 81], F32, name="tnL", tag="tnL")
        _crf_combine_lin(nc, nxtL[:64].unsqueeze(1),
                         cur_ap[:64].unsqueeze(1), bL[:64].unsqueeze(1),
                         crfs, 64, 1)
        logc = crf.tile([P, 81], F32, name="logc", tag="logc")
        nc.scalar.activation(logc[:64, :], nxtL[:64, :], AF.Ln)

        # remaining cross-chunk tree (64 -> 2) in log domain, stabilized
        nact = 64
        cur_ap = logc[:]
        while nact > 2:
            half = nact // 2
            bT = crf.tile([P, 81], F32, name=f"tb{nact}", tag=f"tb{nact}")
            nc.sync.dma_start(bT[:half, :], cur_ap[half:nact])
            nxt = crf.tile([P, 81], F32, name=f"tn{nact}", tag=f"tn{nact}")
            _crf_combine(nc,
                         nxt[:half].unsqueeze(1),
                         cur_ap[:half].unsqueeze(1),
                         bT[:half].unsqueeze(1),
                         crfs, half, 1)
            cur_ap = nxt[:]
            nact = half

        # alpha0 = start + em[:, row 1]; alphaF = alpha0 'logvecmat' Ptot
        a0 = crf.tile([BL, T], F32, name="a0", tag="a0")
        src0 = AP(t["em"].tensor, T, [[S * T, BL], [1, T]])
        nc.sync.dma_start(a0[:], src0)
        st2 = crf.tile([BL, T], F32, name="st2", tag="st2")
        nc.sync.dma_start(st2[:], t["start2"][:])
        nc.vector.tensor_add(a0[:], a0[:], st2[:])

        s0 = crf.tile([BL, T, T], F32, name="s0", tag="s0")   # [b, j, k]
        a0v = a0[:].unsqueeze(1).broadcast_to([BL, T, T])          # k inner
        pv = cur_ap[:BL].rearrange("p (k j) -> p k j", k=T)
        pv = pv.transpose([0, 2, 1])                               # [b, j, k]
        nc.vector.tensor_tensor(out=s0[:], in0=a0v, in1=pv, op=ALU.add)
        mx0 = crf.tile([BL, T], F32, name="mx0", tag="mx0")
        nc.vector.reduce_max(out=mx0[:], in_=s0[:], axis=AX.X)
        mx0v = mx0[:].unsqueeze(2).broadcast_to([BL, T, T])
        nc.vector.tensor_tensor(out=s0[:], in0=s0[:], in1=mx0v,
                                op=ALU.subtract)
        nc.scalar.activation(s0[:], s0[:], AF.Exp)
        sm0 = crf.tile([BL, T], F32, name="sm0", tag="sm0")
        nc.vector.reduce_sum(out=sm0[:], in_=s0[:], axis=AX.X)
        nc.scalar.activation(sm0[:], sm0[:], AF.Ln)
        af = crf.tile([BL, T], F32, name="af", tag="af")
        nc.vector.tensor_add(af[:], sm0[:], mx0[:])
        en2 = crf.tile([BL, T], F32, name="en2", tag="en2")
        nc.sync.dma_start(en2[:], t["end2"][:])
        nc.vector.tensor_add(af[:], af[:], en2[:])
        mx1 = crf.tile([BL, 1], F32, name="mx1", tag="mx1")
        nc.vector.reduce_max(out=mx1[:], in_=af[:], axis=AX.X)
        nc.vector.tensor_scalar(out=af[:], in0=af[:], scalar1=mx1[:, :1],
                                scalar2=None, op0=ALU.subtract)
        nc.scalar.activation(af[:], af[:], AF.Exp)
        sm1 = crf.tile([BL, 1], F32, name="sm1", tag="sm1")
        nc.vector.reduce_sum(out=sm1[:], in_=af[:], axis=AX.X)
        nc.scalar.activation(sm1[:], sm1[:], AF.Ln)
        lz = crf.tile([BL, 1], F32, name="lz", tag="lz")
        nc.vector.tensor_add(lz[:], sm1[:], mx1[:])
        nc.sync.dma_start(t["logz"][:], lz[:])


# ----------------------------------------------------------------------------
# host side
# ----------------------------------------------------------------------------

_NC_CACHE = None
last_exec_time_ns = None
last_results = None


def _get_nc():
    global _NC_CACHE
    if _NC_CACHE is None:
        _NC_CACHE = build_program()
    return _NC_CACHE


def _pairw(w):
    """[Din, Dout] -> [Din//256, 128, 2, Dout] DoubleRow pair layout."""
    din, dout = w.shape
    return np.ascontiguousarray(
        w.reshape(din // 256, 2, P, dout).transpose(0, 2, 1, 3))


def _prep_inputs(inputs):
    """Build the 8 per-core input maps (numpy only)."""
    f8 = ml_dtypes.float8_e4m3
    bf = ml_dtypes.bfloat16
    f32 = np.float32
    x = np.asarray(inputs["x"]).astype(np.int64)
    y = np.asarray(inputs["y"]).astype(np.int64)
    g = {k: np.asarray(v).astype(f32) for k, v in inputs.items()
         if k not in ("x", "y")}

    shared = {}
    shared["wemb"] = g["word_emb"]
    shared["pos"] = g["pos_emb"]

    wqkv8 = np.empty((L, KP, P, 2, 3 * D), f8)
    qkbT = np.empty((L, 96, 16), np.float32)
    vbB = np.empty((L, 1, D), np.float32)
    for l in range(L):
        wq = g["Wqkv"][l][:, :D][:, _PQK]
        wk = g["Wqkv"][l][:, D:2 * D][:, _PQK]
        wv = g["Wqkv"][l][:, 2 * D:]
        wl = np.concatenate([wq, wk, wv], axis=1) * WS
        wqkv8[l] = _pairw(wl.astype(f8))
        bq = g["bqkv"][l][:D][_PQK]
        bk = g["bqkv"][l][D:2 * D][_PQK]
        qkbT[l] = (np.concatenate([bq, bk]) * WS).reshape(16, 96).T
        vbB[l, 0] = g["bqkv"][l][2 * D:] * WS
    shared["wqkv8"] = wqkv8
    shared["qkbT"] = qkbT
    shared["vbB"] = vbB.astype(bf)
    shared["wo8"] = np.stack([_pairw((g["Wo"][l] * WS).astype(f8))
                              for l in range(L)])
    shared["w18"] = np.stack([_pairw((g["W1"][l] * WS).astype(f8))
                              for l in range(L)])
    shared["w28"] = np.stack([_pairw((g["W2"][l] * WS).astype(f8))
                              for l in range(L)])
    wtp = np.zeros((D, 32), np.float32)
    wtp[:, :T] = g["W_tag"] * WS
    shared["wtag8"] = _pairw(wtp.astype(f8))
    shared["boB"] = (g["bo"][:, None, :] * WS).astype(bf)
    shared["b2B"] = (g["b2"][:, None, :] * WS).astype(bf)
    shared["b1T"] = np.ascontiguousarray(
        g["b1"].reshape(L, FF // P, P).transpose(0, 2, 1))
    lnsT = np.stack([g["ln_e_s"]] + [g[f"ln{i}_s"][l] for l in range(L)
                                     for i in (1, 2)])
    lnbT = np.stack([g["ln_e_b"]] + [g[f"ln{i}_b"][l] for l in range(L)
                                     for i in (1, 2)])
    shared["lnsT"] = np.ascontiguousarray(
        lnsT.reshape(2 * L + 1, KD, P).transpose(0, 2, 1))
    shared["lnbT"] = np.ascontiguousarray(
        lnbT.reshape(2 * L + 1, KD, P).transpose(0, 2, 1))
    shared["btag"] = g["b_tag"].reshape(T, 1).copy()
    trans = g["crf_trans"]
    shared["transB"] = np.broadcast_to(trans.reshape(1, 81), (P, 81)).copy()
    ilog = np.full((T, T), NEG, f32)
    np.fill_diagonal(ilog, 0.0)
    shared["ilogB"] = np.broadcast_to(ilog.reshape(1, 81), (P, 81)).copy()
    shared["start2"] = np.broadcast_to(g["crf_start"], (BL, T)).copy()
    shared["permC"] = _BITREV7.reshape(P, 1).astype(np.int32)
    shared["end2"] = np.broadcast_to(g["crf_end"], (BL, T)).copy()

    in_maps = []
    num_consts = []
    for c in range(NCORES):
        xs = x[c * BL:(c + 1) * BL]           # [BL, S]
        ys = y[c * BL:(c + 1) * BL]
        m = {}
        m.update(shared)
        m["tok"] = np.ascontiguousarray(
            xs.reshape(NTOK, 1).astype(np.int32))

        tags = ys[:, 1:]                       # [BL, 511]
        mask = (tags > 0)
        mf = mask.astype(f32)
        mrow = np.zeros((BL, CCH * G), f32)
        mrow[:, :NSTEP] = mf[:, 1:]
        m["maskB"] = np.ascontiguousarray(
            mrow.reshape(BL * CCH, G)[_BITREV7])
        sel = np.zeros((BL, S, T), f32)
        bi = np.arange(BL)[:, None]
        tpos = np.arange(S - 1)[None, :]
        w = np.concatenate([np.ones((BL, 1), f32), mf[:, 1:]], axis=1)
        sel[bi, tpos + 1, tags] = w
        m["selT"] = np.ascontiguousarray(sel.reshape(NTOK, T).T)
        in_maps.append(m)

        tr = trans[tags[:, :-1], tags[:, 1:]]
        num_c = g["crf_start"][tags[:, 0]].sum()
        num_c += (tr * mf[:, 1:]).sum()
        last = mask.sum(axis=1).astype(np.int64) - 1
        num_c += g["crf_end"][tags[np.arange(BL), last]].sum()
        num_consts.append(float(num_c))
    return in_maps, num_consts


def kernel(**inputs):
    global last_exec_time_ns
    import os
    nc = _get_nc()
    in_maps, num_consts = _prep_inputs(inputs)
    trace = bool(int(os.environ.get("KERNEL_TRACE", "0")))
    if trace:
        import concourse.bass_utils as _BU
        _BU.upload_artifacts = lambda tmpdir: tmpdir
        tdir = os.environ.get("KERNEL_TRACE_DIR")
        if tdir:
            os.makedirs(tdir, exist_ok=True)
        try:
            res = run_bass_kernel_spmd(
                nc, in_maps, core_ids=list(range(NCORES)), trace=True,
                tmpdir=tdir)
            global last_results
            last_results = res
        except Exception as e:
            print(f"trace run failed ({e!r}); retrying untraced")
            res = run_bass_kernel_spmd(
                nc, in_maps, core_ids=list(range(NCORES)), trace=False)
    else:
        res = run_bass_kernel_spmd(
            nc, in_maps, core_ids=list(range(NCORES)), trace=False)
    last_exec_time_ns = res.exec_time_ns
    loss = 0.0
    for c in range(NCORES):
        r = res.results[c]
        num = num_consts[c] + float(r["numdot"].sum())
        logz = float(r["logz"].sum())
        loss += logz - num
    return np.float32(loss)


# revision 44
# speedup vs baseline: 1.3192x; 1.3192x over previous
"""BERT(2-layer) + CRF NLL loss kernel for Trainium2, data-parallel over batch on 8 cores.

fp8 (e4m3) DoubleRow matmuls for all linear layers (halves PSUM accumulation
passes: K=256 per instruction), bf16 pre-LN accumulators, fp8 residual stream
(scale 1, weights x64), DVE-only rsqrt for LN (bit-trick seed + 1 Newton step,
768-scale folded into the Newton constants -- no Ln/Exp act-table loads on the
LN critical path), QK bias folded into the PSUM-drain tensor_scalar.

Layout per core (2 examples, 1024 token-slots), all feature-major:
  - h8   fp8 [128, 2, 1024] x3: post-LN activations, pair i = feature k-tile 2p+i.
  - q8/k8 fp8 [96, 2, 1024] x4: tile a holds heads 3a..3a+2 at partition bases
    0/32/64 (PE cannot read base 96), pair i = dh 32i+r, via a host-side
    wqkv column permutation; scores contract DH=64 as [32,2] DoubleRow.
  - v8 fp8 [128, 2, 768] x4 token-groups (token (2u+i)*128+p); softmax
    denominator via a separate ones[128,2,32] DoubleRow matmul (DoubleRow
    stationary must be >=32 columns).
  - ctx8 fp8 x3 natural feature pairs; xres bf16 x6: pre-LN residual accumulators.
  - weights fp8 x64 in DoubleRow pair layout [128, 2, out]; Wo/W2/V biases via
    small bf16 rank-1 matmuls (64*b rows x ones) accumulated into PSUM; W1
    bias via the gelu activation bias; QK bias via per-partition tensor_scalar.
  - FF runs in 256-token chunks so W1/gelu/W2 pipeline inside 8 PSUM banks.
  - CRF identical to v1 (log-domain associative scan over 9x9 matrices).

Known hw caveats found on the way: DoubleRow is 1 cycle/row on TRN2 (cost
model says 0.5) -- the win is 2x K per instruction, not faster rows; the
chip power-throttles the PE to ~50% duty for ~2/3 of this kernel (dense
PE-only streams do not trip it); tensor_tensor_reduce crashes the device;
engine ops need partition bases in {0,32,64,96}.
"""

import sys

sys.path.insert(0, "/opt/trn_rl_repo")

import numpy as np
import ml_dtypes

import concourse.bass as bass
import concourse.tile as tile
from concourse import bacc, mybir
from concourse.bass import AP
from concourse.bass_utils import run_bass_kernel_spmd
from concourse.masks import make_identity

F32 = mybir.dt.float32
BF16 = mybir.dt.bfloat16
FP8 = mybir.dt.float8e4
I32 = mybir.dt.int32
AF = mybir.ActivationFunctionType
ALU = mybir.AluOpType
AX = mybir.AxisListType
DR = mybir.MatmulPerfMode.DoubleRow

P = 128
B, S, D, L, H, T, V = 16, 512, 768, 2, 12, 9, 30522
DH = D // H          # 64
FF = 4 * D           # 3072
NCORES = 8
BL = B // NCORES     # 2 examples per core
NTOK = BL * S        # 1024
KD = D // P          # 6 k-tiles over D
KP = KD // 2         # 3 k-pair tiles
FFP = FF // 256      # 12 ff-pair tiles
NT = NTOK // 512     # 2 n-chunks of 512 tokens
TT = NTOK // P       # 8 token-tiles
EPS = 1e-12
NEG = -1000.0        # effective -inf for log-domain CRF
G = 8                # CRF scan steps per chunk
CCH = 64             # chunks per example
NSTEP = 510          # scan steps (S'-1 where S'=511)
EMROWS = NTOK + 16   # em output padded so chunk loads never go OOB
WS = 64.0            # fp8 weight scale
IWS = 1.0 / WS
ES = 16.0            # exp tile scale
LN16 = float(np.log(ES))
LN768 = float(np.log(768.0))
EPS_S = 768.0 * 768.0 * EPS

def _bitrev(n, bits):
    r = 0
    for _ in range(bits):
        r = (r << 1) | (n & 1)
        n >>= 1
    return r

_BITREV7 = np.array([_bitrev(p, 7) for p in range(128)], dtype=np.int64)

# q/k output-feature permutation: tile a holds heads 3a..3a+2 on partitions
# 32j+r (j=head%3, base 0/32/64 only -- PE cannot read from base 96), pair i
# selects dh 32i+r. Permuted column a*192 + i*96 + j*32 + r <- head 3a+j,
# dh i*32+r.
_PQK = np.empty(D, dtype=np.int64)
for _a in range(4):
    for _i in range(2):
        for _j in range(3):
            for _r in range(32):
                _PQK[_a * 192 + _i * 96 + _j * 32 + _r] = \
                    (3 * _a + _j) * 64 + _i * 32 + _r


# ----------------------------------------------------------------------------
# device program
# ----------------------------------------------------------------------------

def build_program():
    nc = bacc.Bacc("TRN2", target_bir_lowering=False, debug=False)

    def din(name, shape, dt):
        return nc.dram_tensor(name, shape, dt, kind="ExternalInput").ap()

    def dout(name, shape, dt):
        return nc.dram_tensor(name, shape, dt, kind="ExternalOutput").ap()

    t = dict(
        tok=din("tok", [NTOK, 1], I32),
        wemb=din("wemb", [V, D], F32),
        pos=din("pos", [S, D], F32),
        wqkv8=din("wqkv8", [L, KP, P, 2, 3 * D], FP8),
        wo8=din("wo8", [L, KP, P, 2, D], FP8),
        w18=din("w18", [L, KP, P, 2, FF], FP8),
        w28=din("w28", [L, FFP, P, 2, D], FP8),
        wtag8=din("wtag8", [KP, P, 2, 32], FP8),
        qkbT=din("qkbT", [L, 96, 16], F32),      # 64*b, permuted, per-tile cols
        vbB=din("vbB", [L, 1, D], BF16),         # 64*b_v
        boB=din("boB", [L, 1, D], BF16),
        b2B=din("b2B", [L, 1, D], BF16),
        b1T=din("b1T", [L, P, FF // P], F32),
        lnsT=din("lnsT", [2 * L + 1, P, KD], F32),
        lnbT=din("lnbT", [2 * L + 1, P, KD], F32),
        btag=din("btag", [T, 1], F32),
        transB=din("transB", [P, 81], F32),
        ilogB=din("ilogB", [P, 81], F32),
        maskB=din("maskB", [P, G], F32),
        start2=din("start2", [BL, T], F32),
        end2=din("end2", [BL, T], F32),
        selT=din("selT", [T, NTOK], F32),
        permC=din("permC", [P, 1], I32),
        emS=nc.dram_tensor("emS", [P, G * T], F32, kind="Internal").ap(),
        em=dout("em", [EMROWS, T], F32),
        numdot=dout("numdot", [T, 1], F32),
        logz=dout("logz", [BL, 1], F32),
    )

    with tile.TileContext(nc) as tc:
        _emit(nc, tc, t)
    nc.compile()
    return nc


def _emit(nc, tc, t):
    from contextlib import ExitStack

    with ExitStack() as ctx:
        const = ctx.enter_context(tc.tile_pool(name="const", bufs=1))
        hpool = ctx.enter_context(tc.tile_pool(name="h", bufs=1))
        wpool = ctx.enter_context(tc.tile_pool(name="w", bufs=1))

        ident = const.tile([P, P], F32, name="ident", tag="ident")
        make_identity(nc, ident[:])
        identb = const.tile([P, P], BF16, name="identb", tag="identb")
        make_identity(nc, identb[:])
        ones_row = const.tile([1, 512], BF16, name="ones_row", tag="ones_row")
        nc.vector.memset(ones_row[:], 1.0)
        ones1b = const.tile([1, P], BF16, name="ones1b", tag="ones1b")
        nc.vector.memset(ones1b[:], 1.0)
        c768row = const.tile([1, P], BF16, name="c768row", tag="c768row")
        nc.vector.memset(c768row[:], 1.0 / 768.0)
        ones128b = const.tile([P, 1], BF16, name="ones128b", tag="ones128b")
        nc.vector.memset(ones128b[:], 1.0)
        epsS = const.tile([1, 1], F32, name="epsS", tag="epsS")
        nc.vector.memset(epsS[:], EPS_S)
        ln768t = const.tile([1, 1], F32, name="ln768t", tag="ln768t")
        nc.vector.memset(ln768t[:], LN768)
        ln16t = const.tile([P, 1], F32, name="ln16t", tag="ln16t")
        nc.vector.memset(ln16t[:], LN16)
        cbias = dict(epsS=epsS, ln768t=ln768t, ln16t=ln16t)  # + ones8 below

        # persistent activation tiles
        h8 = [hpool.tile([P, 2, NTOK], FP8, name=f"h8_{p}", tag=f"h8_{p}")
              for p in range(KP)]
        q8 = [hpool.tile([96, 2, NTOK], FP8, name=f"q8_{a}", tag=f"q8_{a}")
              for a in range(4)]
        k8 = [hpool.tile([96, 2, NTOK], FP8, name=f"k8_{a}", tag=f"k8_{a}")
              for a in range(4)]
        v8 = [hpool.tile([P, 2, H * DH], FP8, name=f"v8_{u}", tag=f"v8_{u}")
              for u in range(4)]
        ones8 = const.tile([P, 2, 32], FP8, name="ones8", tag="ones8")
        nc.vector.memset(ones8[:], 1.0)
        cbias["ones8"] = ones8
        ctx8 = [hpool.tile([P, 2, NTOK], FP8, name=f"ctx8_{p}", tag=f"ctx8_{p}")
                for p in range(KP)]
        xres = [hpool.tile([P, NTOK], BF16, name=f"xres{k}", tag=f"xres{k}")
                for k in range(KD)]

        # LN scale/bias param tiles ([P, site, k])
        lns = const.tile([P, 2 * L + 1, KD], F32, name="lns", tag="lns")
        nc.sync.dma_start(lns[:], t["lnsT"].rearrange("a p k -> p a k"))
        lnb = const.tile([P, 2 * L + 1, KD], F32, name="lnb", tag="lnb")
        nc.sync.dma_start(lnb[:], t["lnbT"].rearrange("a p k -> p a k"))

        # ------------------------------------------------------------------
        # embedding: gather + pos, cast bf16, transpose to xres, then LN
        # ------------------------------------------------------------------
        with tc.tile_pool(name="emb", bufs=2) as emb, \
             tc.tile_pool(name="embps", bufs=3, space="PSUM") as embps, \
             tc.tile_pool(name="posp", bufs=1) as posp:
            pos_t = []
            for q in range(S // P):
                pt = posp.tile([P, D], F32, name=f"pos{q}", tag=f"pos{q}")
                nc.sync.dma_start(pt[:], t["pos"][q * P:(q + 1) * P, :])
                pos_t.append(pt)
            for n in range(NT):
                gbfs = []
                for q in range(4):
                    tt_i = n * 4 + q
                    idx = emb.tile([P, 1], I32, name="idx", tag="idx")
                    nc.sync.dma_start(idx[:], t["tok"][tt_i * P:(tt_i + 1) * P, :])
                    g32 = emb.tile([P, D], F32, name="g32", tag="g32")
                    nc.gpsimd.indirect_dma_start(
                        out=g32[:], out_offset=None, in_=t["wemb"][:],
                        in_offset=bass.IndirectOffsetOnAxis(ap=idx[:, :1], axis=0),
                    )
                    gbf = emb.tile([P, D], BF16, name=f"gbf{q}", tag=f"gbf{q}")
                    nc.vector.tensor_add(gbf[:], g32[:], pos_t[tt_i % 4][:])
                    gbfs.append(gbf)
                for k in range(KD):
                    pb = embps.tile([P, 4, P], BF16, name="pb", tag="pb",
                                    space="PSUM")
                    for q in range(4):
                        nc.tensor.transpose(
                            pb[:, q, :], gbfs[q][:, k * P:(k + 1) * P], identb[:])
                    nc.vector.tensor_copy(
                        xres[k][:, n * 512:(n + 1) * 512],
                        pb[:].rearrange("p q c -> p (q c)"))
        for n in range(NT):
            _ln_apply(nc, tc, n, xres, h8, lns[:, 0, :], lnb[:, 0, :],
                      ones128b, ones1b, c768row, cbias)

        # ------------------------------------------------------------------
        # weights to SBUF (fp8)
        # ------------------------------------------------------------------
        wq8 = [[wpool.tile([P, 2, 3 * D], FP8, name=f"wq{l}_{p}", tag=f"wq{l}_{p}")
                for p in range(KP)] for l in range(L)]
        wo8 = [[wpool.tile([P, 2, D], FP8, name=f"wo{l}_{p}", tag=f"wo{l}_{p}")
                for p in range(KP)] for l in range(L)]
        w18 = [[wpool.tile([P, 2, FF], FP8, name=f"w1{l}_{p}", tag=f"w1{l}_{p}")
                for p in range(KP)] for l in range(L)]
        w28 = [[wpool.tile([P, 2, D], FP8, name=f"w2{l}_{f}", tag=f"w2{l}_{f}")
                for f in range(FFP)] for l in range(L)]
        for l in range(L):
            for p in range(KP):
                nc.sync.dma_start(wq8[l][p][:], t["wqkv8"][l, p])
                nc.sync.dma_start(wo8[l][p][:], t["wo8"][l, p])
                nc.sync.dma_start(w18[l][p][:], t["w18"][l, p])
            for f in range(FFP):
                nc.sync.dma_start(w28[l][f][:], t["w28"][l, f])
        qkb = const.tile([96, L, 16], F32, name="qkb", tag="qkb")
        nc.sync.dma_start(qkb[:], t["qkbT"].rearrange("l p m -> p l m"))
        vbb = const.tile([1, L, D], BF16, name="vbb", tag="vbb")
        nc.sync.dma_start(vbb[:], t["vbB"].rearrange("l o d -> o l d"))
        bob = const.tile([1, L, D], BF16, name="bob", tag="bob")
        nc.sync.dma_start(bob[:], t["boB"].rearrange("l o d -> o l d"))
        b2b = const.tile([1, L, D], BF16, name="b2b", tag="b2b")
        nc.sync.dma_start(b2b[:], t["b2B"].rearrange("l o d -> o l d"))
        b1t = const.tile([P, L, FF // P], F32, name="b1t", tag="b1t")
        nc.sync.dma_start(b1t[:], t["b1T"].rearrange("l p k -> p l k"))

        # ------------------------------------------------------------------
        # encoder layers
        # ------------------------------------------------------------------
        for l in range(L):
            _layer(nc, tc, t, l, h8, q8, k8, v8, ctx8, xres,
                   wq8[l], wo8[l], w18[l], w28[l],
                   qkb, vbb[:, l, :], bob[:, l, :], b2b[:, l, :],
                   b1t[:, l, :],
                   lns, lnb, ones_row, ones1b, c768row, ones128b, cbias)

        # ------------------------------------------------------------------
        # emissions: em = wtag.T @ h + btag  (feature-major [9, NTOK])
        # ------------------------------------------------------------------
        with tc.tile_pool(name="emp", bufs=1) as emp, \
             tc.tile_pool(name="emps", bufs=2, space="PSUM") as emps:
            wtg = [emp.tile([P, 2, 32], FP8, name=f"wtg{p}", tag=f"wtg{p}")
                   for p in range(KP)]
            for p in range(KP):
                nc.sync.dma_start(wtg[p][:], t["wtag8"][p])
            btg = emp.tile([T, 1], F32, name="btg", tag="btg")
            nc.sync.dma_start(btg[:], t["btag"][:])
            em_sb = emp.tile([T, NTOK], F32, name="em_sb", tag="em_sb")
            for n in range(NT):
                ps = emps.tile([32, 512], F32, name="emmm", tag="emmm",
                               space="PSUM")
                for p in range(KP):
                    nc.tensor.matmul(
                        ps[:], lhsT=wtg[p][:],
                        rhs=h8[p][:, :, n * 512:(n + 1) * 512],
                        start=(p == 0), stop=(p == KP - 1), perf_mode=DR)
                nc.scalar.activation(
                    em_sb[:, n * 512:(n + 1) * 512], ps[:T, :], AF.Identity,
                    bias=btg[:, :1], scale=IWS)
            # numerator dot: sum(em * selT) fused multiply-reduce
            sel = emp.tile([T, NTOK], F32, name="sel", tag="sel")
            nc.sync.dma_start(sel[:], t["selT"][:])
            prod = emp.tile([T, NTOK], F32, name="prod", tag="prod")
            nc.vector.tensor_mul(prod[:], em_sb[:], sel[:])
            nd = emp.tile([T, 1], F32, name="nd", tag="nd")
            nc.vector.reduce_sum(out=nd[:], in_=prod[:], axis=AX.X)
            nc.sync.dma_start(t["numdot"][:], nd[:])
            # token-major em to DRAM (+ zero pad rows)
            zpad = emp.tile([16, T], F32, name="zpad", tag="zpad")
            nc.vector.memset(zpad[:], 0.0)
            nc.sync.dma_start(t["em"][NTOK:EMROWS, :], zpad[:])
            for tt_i in range(TT):
                tp = emps.tile([P, T], F32, name="emtp", tag="emtp", space="PSUM")
                nc.tensor.transpose(
                    tp[:], em_sb[:, tt_i * P:(tt_i + 1) * P], ident[:T, :T])
                emtm = emp.tile([P, T], F32, name="emtm", tag="emtm", bufs=3)
                nc.vector.tensor_copy(emtm[:], tp[:])
                nc.sync.dma_start(t["em"][tt_i * P:(tt_i + 1) * P, :], emtm[:])

        # ------------------------------------------------------------------
        # CRF forward pass (log-domain associative scan)
        # ------------------------------------------------------------------
        _crf(nc, tc, t)


def _ln_apply(nc, tc, n, xres, h8, sT, bT, ones128b, ones1b, c768row, cbias):
    """Feature-major LN of xres (bf16) chunk n -> h8 (fp8).

    rstd computed as exp(-0.5 ln(768*sq - mu^2) + ln 768).
    """
    sl = slice(n * 512, (n + 1) * 512)
    with tc.tile_pool(name="lnp", bufs=1) as lnp, \
         tc.tile_pool(name="lnps", bufs=1, space="PSUM") as lnps:
        mu_ps = lnps.tile([1, 512], F32, name="mu", tag="mu", space="PSUM")
        sq_ps = lnps.tile([1, 512], F32, name="sq", tag="sq", space="PSUM")
        xsq = [lnp.tile([P, 512], BF16, name=f"xsq{k}", tag=f"xsq{k}", bufs=1)
               for k in range(KD)]
        for k in range(KD):
            nc.vector.tensor_mul(xsq[k][:], xres[k][:, sl], xres[k][:, sl])
        for k in range(KD):
            nc.tensor.matmul(mu_ps[:], lhsT=ones128b[:], rhs=xres[k][:, sl],
                             start=(k == 0), stop=(k == KD - 1))
        for k in range(KD):
            nc.tensor.matmul(sq_ps[:], lhsT=ones128b[:], rhs=xsq[k][:],
                             start=(k == 0), stop=(k == KD - 1))
        musq = lnp.tile([1, 512], F32, name="musq", tag="musq")
        nc.scalar.square(musq[:], mu_ps[:])
        svar = lnp.tile([1, 512], F32, name="svar", tag="svar")
        nc.vector.scalar_tensor_tensor(
            out=svar[:], in0=sq_ps[:], scalar=768.0, in1=musq[:],
            op0=ALU.mult, op1=ALU.subtract)
        # rstd = 768*rsqrt(svar) via bit-trick seed + one Newton step, all
        # on DVE (no Ln/Exp -> no act-table loads on the LN critical path)
        q1 = lnp.tile([1, 512], I32, name="q1", tag="q1")
        nc.vector.tensor_scalar(
            out=q1[:], in0=svar[:].bitcast(I32), scalar1=1, scalar2=None,
            op0=ALU.logical_shift_right)
        q2 = lnp.tile([1, 512], I32, name="q2", tag="q2")
        nc.vector.tensor_scalar(
            out=q2[:], in0=q1[:], scalar1=-1, scalar2=None,
            op0=ALU.bitwise_xor)
        y0 = lnp.tile([1, 512], F32, name="y0", tag="y0")
        nc.vector.tensor_scalar(
            out=y0[:].bitcast(I32), in0=q2[:], scalar1=0x5F3759E0,
            scalar2=None, op0=ALU.add)
        n1 = lnp.tile([1, 512], F32, name="n1", tag="n1")
        nc.vector.tensor_mul(n1[:], y0[:], y0[:])
        n2 = lnp.tile([1, 512], F32, name="n2", tag="n2")
        nc.vector.tensor_mul(n2[:], n1[:], svar[:])
        n3 = lnp.tile([1, 512], F32, name="n3", tag="n3")
        nc.vector.tensor_scalar(
            out=n3[:], in0=n2[:], scalar1=-384.0, scalar2=1152.0,
            op0=ALU.mult, op1=ALU.add)
        rs = lnp.tile([1, 512], BF16, name="rs", tag="rs")
        nc.vector.tensor_mul(rs[:], y0[:], n3[:])
        murs = lnp.tile([1, 512], BF16, name="murs", tag="murs")
        nc.vector.tensor_mul(murs[:], mu_ps[:], rs[:])
        rsB_ps = lnps.tile([P, 512], F32, name="rsB", tag="rsB", space="PSUM")
        nc.tensor.matmul(rsB_ps[:], lhsT=ones1b[:], rhs=rs[:],
                         start=True, stop=True)
        m2_ps = lnps.tile([P, 512], F32, name="m2B", tag="m2B", space="PSUM")
        nc.tensor.matmul(m2_ps[:], lhsT=c768row[:], rhs=murs[:],
                         start=True, stop=True)
        rsB = lnp.tile([P, 512], BF16, name="rsBs", tag="rsBs")
        nc.vector.tensor_copy(rsB[:], rsB_ps[:])
        m2B = lnp.tile([P, 512], BF16, name="m2Bs", tag="m2Bs")
        nc.vector.tensor_copy(m2B[:], m2_ps[:])
        for k in range(KD):
            tm = lnp.tile([P, 512], BF16, name="tm", tag="tm", bufs=3)
            nc.vector.tensor_mul(tm[:], xres[k][:, sl], rsB[:])
            ts = lnp.tile([P, 512], BF16, name="ts", tag="ts", bufs=3)
            nc.vector.tensor_sub(ts[:], tm[:], m2B[:])
            dst = h8[k // 2][:, k % 2, sl]
            nc.scalar.activation(dst, ts[:], AF.Identity,
                                 bias=bT[:, k:k + 1], scale=sT[:, k:k + 1])


def _layer(nc, tc, t, l, h8, q8, k8, v8, ctx8, xres,
           wq8, wo8, w18, w28, qkb, vbb, bob, b2b, b1t,
           lns, lnb, ones_row, ones1b, c768row, ones128b, cbias):
    # --------------- QKV: q8/k8 feature-major perm, v8 token-major ---------
    with tc.tile_pool(name="qkps", bufs=4, space="PSUM") as qkps:
        for n in range(NT):
            nsl = slice(n * 512, (n + 1) * 512)
            for m in range(16):       # 8 Q tiles then 8 K tiles (96-part, perm)
                csl = slice(m * 96, (m + 1) * 96)
                ps = qkps.tile([96, 512], F32, name="ps", tag="ps", space="PSUM")
                for p in range(KP):
                    nc.tensor.matmul(
                        ps[:], lhsT=wq8[p][:, :, csl],
                        rhs=h8[p][:, :, nsl],
                        start=(p == 0), stop=(p == KP - 1), perf_mode=DR,
                        skip_group_check=True)
                dstq = q8 if m < 8 else k8
                mm = m % 8
                nc.vector.tensor_scalar(
                    out=dstq[mm // 2][:, mm % 2, nsl], in0=ps[:],
                    scalar1=qkb[:, l, m:m + 1], scalar2=IWS,
                    op0=ALU.add, op1=ALU.mult)
        for tt_i in range(TT):        # V token-major
            for nv in range(2):
                vsl = slice(2 * D + nv * 384, 2 * D + (nv + 1) * 384)
                ps = qkps.tile([P, 384], F32, name="psv", tag="psv", space="PSUM")
                nc.tensor.matmul(ps[:], lhsT=ones1b[:],
                                 rhs=vbb[:, nv * 384:(nv + 1) * 384],
                                 start=True, stop=False, skip_group_check=True)
                for p in range(KP):
                    nc.tensor.matmul(
                        ps[:], lhsT=h8[p][:, :, tt_i * P:(tt_i + 1) * P],
                        rhs=wq8[p][:, :, vsl],
                        start=False, stop=(p == KP - 1), perf_mode=DR,
                        skip_group_check=True)
                nc.vector.tensor_scalar_mul(
                    v8[tt_i // 2][:, tt_i % 2, nv * 384:(nv + 1) * 384],
                    ps[:], IWS)

    # --------------- attention ----------------------------------------
    with tc.tile_pool(name="att", bufs=1) as att, \
         tc.tile_pool(name="scps", bufs=2, space="PSUM") as scps, \
         tc.tile_pool(name="ctxps", bufs=2, space="PSUM") as ctxps, \
         tc.tile_pool(name="denps", bufs=1, space="PSUM") as denps, \
         tc.tile_pool(name="invps", bufs=1, space="PSUM") as invps:
        for b in range(BL):
            bsl = slice(b * S, (b + 1) * S)
            for hp in range(H // 2):
                cps = []
                ivs = []
                for hh in range(2):
                    h = hp * 2 + hh
                    a, j = h // 3, h % 3
                    psl = slice(32 * j, 32 * j + 32)
                    cp = ctxps.tile([DH, 512], F32, name="ctx", tag="ctx",
                                    space="PSUM")
                    den = denps.tile([32, 512], F32, name="den", tag="den",
                                     space="PSUM")
                    for u in range(2):
                        sc = scps.tile([P, 2, 512], F32, name="sc", tag="sc",
                                       space="PSUM")
                        for i in range(2):
                            kt = 2 * u + i
                            nc.tensor.matmul(
                                sc[:, i, :],
                                lhsT=k8[a][psl, :,
                                           b * S + kt * P:b * S + (kt + 1) * P],
                                rhs=q8[a][psl, :, bsl],
                                start=True, stop=True, perf_mode=DR)
                        et = att.tile([P, 2, 512], FP8, name="et", tag="et",
                                      bufs=4)
                        nc.scalar.activation(
                            et[:].rearrange("p i q -> p (i q)"),
                            sc[:].rearrange("p i q -> p (i q)"),
                            AF.Exp, scale=0.125, bias=cbias["ln16t"][:, :1])
                        nc.tensor.matmul(
                            cp[:],
                            lhsT=v8[2 * b + u][:, :, h * DH:(h + 1) * DH],
                            rhs=et[:], start=(u == 0), stop=(u == 1),
                            perf_mode=DR, skip_group_check=True)
                        nc.tensor.matmul(
                            den[:], lhsT=cbias["ones8"][:], rhs=et[:],
                            start=(u == 0), stop=(u == 1),
                            perf_mode=DR, skip_group_check=True)
                    iv = att.tile([1, 512], F32, name="iv", tag="iv", bufs=4)
                    nc.vector.reciprocal_approx_fast(iv[:], den[:1, :])
                    ivb = att.tile([1, 512], BF16, name="ivb", tag="ivb",
                                   bufs=4)
                    nc.vector.tensor_copy(ivb[:], iv[:])
                    cps.append(cp)
                    ivs.append(ivb)
                ivB = invps.tile([P, 512], F32, name="ivB", tag="ivB",
                                 space="PSUM")
                nc.tensor.matmul(ivB[:DH, :], lhsT=ones1b[:, :DH],
                                 rhs=ivs[0][:], start=True, stop=True)
                nc.tensor.matmul(ivB[DH:, :], lhsT=ones1b[:, :DH],
                                 rhs=ivs[1][:], start=True, stop=True)
                ivS = att.tile([P, 512], BF16, name="ivS", tag="ivS", bufs=2)
                nc.scalar.copy(ivS[:], ivB[:])
                for hh in range(2):
                    h = hp * 2 + hh
                    po = (h % 2) * DH
                    nc.vector.tensor_mul(
                        ctx8[h // 4][po:po + DH, (h // 2) % 2, bsl],
                        cps[hh][:, :], ivS[po:po + DH, :])

    # --------------- Wo + residual + LN1 -------------------------------
    with tc.tile_pool(name="wops", bufs=3, space="PSUM") as wops:
        for n in range(NT):
            nsl = slice(n * 512, (n + 1) * 512)
            for m in range(KD):
                ps = wops.tile([P, 512], F32, name="ps", tag="ps", space="PSUM")
                nc.tensor.matmul(ps[:], lhsT=bob[:, m * P:(m + 1) * P],
                                 rhs=ones_row[:], start=True, stop=False,
                                 skip_group_check=True)
                for p in range(KP):
                    nc.tensor.matmul(
                        ps[:], lhsT=wo8[p][:, :, m * P:(m + 1) * P],
                        rhs=ctx8[p][:, :, nsl],
                        start=False, stop=(p == KP - 1), perf_mode=DR,
                        skip_group_check=True)
                nc.vector.scalar_tensor_tensor(
                    out=xres[m][:, nsl], in0=ps[:], scalar=IWS,
                    in1=h8[m // 2][:, m % 2, nsl], op0=ALU.mult, op1=ALU.add)
            _ln_apply(nc, tc, n, xres, h8, lns[:, 2 * l + 1, :],
                      lnb[:, 2 * l + 1, :], ones128b, ones1b, c768row, cbias)

    # --------------- FF (256-token chunks, pipelined in PSUM) -----------
    with tc.tile_pool(name="ffac", bufs=1, space="PSUM") as ffac, \
         tc.tile_pool(name="ffps", bufs=2, space="PSUM") as ffps, \
         tc.tile_pool(name="ffg", bufs=3) as ffg:
        acc = [ffac.tile([P, 2, 256], F32, name=f"acc{m}", tag=f"acc{m}",
                         space="PSUM") for m in range(KD)]
        for c in range(4):
            csl = slice(c * 256, (c + 1) * 256)
            ci = c % 2
            for m in range(KD):
                nc.tensor.matmul(acc[m][:, ci, :],
                                 lhsT=b2b[:, m * P:(m + 1) * P],
                                 rhs=ones_row[:, :256], start=True, stop=False,
                                 skip_group_check=True)
            for f in range(FFP):
                psg = ffps.tile([P, 2, 256], F32, name="psg", tag="psg",
                                space="PSUM")
                for i in range(2):
                    for p in range(KP):
                        nc.tensor.matmul(
                            psg[:, i, :],
                            lhsT=w18[p][:, :, (2 * f + i) * P:(2 * f + i + 1) * P],
                            rhs=h8[p][:, :, csl],
                            start=(p == 0), stop=(p == KP - 1), perf_mode=DR)
                gl = ffg.tile([P, 2, 256], FP8, name="gl", tag="gl")
                for i in range(2):
                    nc.scalar.activation(
                        gl[:, i, :], psg[:, i, :], AF.Gelu,
                        bias=b1t[:, 2 * f + i:2 * f + i + 1], scale=IWS)
                for m in range(KD):
                    nc.tensor.matmul(
                        acc[m][:, ci, :], lhsT=w28[f][:, :, m * P:(m + 1) * P],
                        rhs=gl[:], start=False, stop=(f == FFP - 1),
                        perf_mode=DR, skip_group_check=True)
            for m in range(KD):
                nc.vector.scalar_tensor_tensor(
                    out=xres[m][:, csl], in0=acc[m][:, ci, :], scalar=IWS,
                    in1=h8[m // 2][:, m % 2, csl], op0=ALU.mult, op1=ALU.add)
    for n in range(NT):
        _ln_apply(nc, tc, n, xres, h8, lns[:, 2 * l + 2, :],
                  lnb[:, 2 * l + 2, :], ones128b, ones1b, c768row, cbias)


def _crf_combine_lin(nc, out_ap, a_ap, b_ap, spool, npart, npair):
    """Linear-domain combine: out[i,j] = sum_k A[i,k]*B[k,j] (no Act ops)."""
    s = spool.tile([P, 4, 729], F32, name="cS", tag="cS")
    sv4 = s[:npart, :npair, :].rearrange("p q (x k) -> p q x k", k=T)
    for q in range(npair):
        avq = a_ap[:, q].rearrange("p (i k) -> p i k", i=T)
        avq = avq.unsqueeze(2).broadcast_to([npart, T, T, T])    # p i j k
        bvq = b_ap[:, q].rearrange("p (k j) -> p k j", k=T)
        bvq = bvq.unsqueeze(1).broadcast_to([npart, T, T, T])    # p i k j
        bvq = bvq.transpose([0, 1, 3, 2])                        # p i j k
        svq = s[:npart, q, :].rearrange("p (i j k) -> p i j k", i=T, j=T)
        nc.vector.tensor_tensor(out=svq, in0=avq, in1=bvq, op=ALU.mult)
    nc.vector.reduce_sum(out=out_ap, in_=sv4, axis=AX.X)


def _crf_combine(nc, out_ap, a_ap, b_ap, spool, npart, npair, stabilize=True):
    """out = A 'logmatmul' B over pairs: out[i,j] = lse_k(A[i,k]+B[k,j])."""
    s = spool.tile([P, 4, 729], F32, name="cS", tag="cS")
    sv4 = s[:npart, :npair, :].rearrange("p q (x k) -> p q x k", k=T)
    sv3 = s[:npart, :npair, :]
    for q in range(npair):
        avq = a_ap[:, q].rearrange("p (i k) -> p i k", i=T)
        avq = avq.unsqueeze(2).broadcast_to([npart, T, T, T])    # p i j k
        bvq = b_ap[:, q].rearrange("p (k j) -> p k j", k=T)
        bvq = bvq.unsqueeze(1).broadcast_to([npart, T, T, T])    # p i k j
        bvq = bvq.transpose([0, 1, 3, 2])                        # p i j k
        svq = s[:npart, q, :].rearrange("p (i j k) -> p i j k", i=T, j=T)
        nc.vector.tensor_tensor(out=svq, in0=avq, in1=bvq, op=ALU.add)
    sm = spool.tile([P, 4, 81], F32, name="cR", tag="cR")
    sm3 = sm[:npart, :npair, :]
    if stabilize:
        mx = spool.tile([P, 4, 81], F32, name="cM", tag="cM")
        mx3 = mx[:npart, :npair, :]
        nc.vector.reduce_max(out=mx3, in_=sv4, axis=AX.X)
        mxv = mx3.unsqueeze(3).broadcast_to([npart, npair, 81, T])
        nc.vector.tensor_tensor(out=sv4, in0=sv4, in1=mxv, op=ALU.subtract)
        nc.scalar.activation(sv3, sv3, AF.Exp)
        nc.vector.reduce_sum(out=sm3, in_=sv4, axis=AX.X)
        nc.scalar.activation(sm3, sm3, AF.Ln)
        nc.vector.tensor_tensor(out=out_ap, in0=sm3, in1=mx3, op=ALU.add)
    else:
        nc.scalar.activation(sv3, sv3, AF.Exp)
        nc.vector.reduce_sum(out=sm3, in_=sv4, axis=AX.X)
        nc.scalar.activation(out_ap, sm3, AF.Ln)


def _crf(nc, tc, t):
    """Log-domain associative scan. Partitions 0..63 = example0 chunks,
    64..127 = example1 chunks; each chunk = G=8 consecutive scan steps."""
    with tc.tile_pool(name="crf", bufs=1) as crf, \
         tc.tile_pool(name="crfs", bufs=1) as crfs:
        transB = crf.tile([P, 81], F32, name="transB", tag="transB")
        nc.sync.dma_start(transB[:], t["transB"][:])
        ilogB = crf.tile([P, 81], F32, name="ilogB", tag="ilogB")
        nc.sync.dma_start(ilogB[:], t["ilogB"][:])
        maskB = crf.tile([P, G], F32, name="maskB", tag="maskB")
        nc.sync.dma_start(maskB[:], t["maskB"][:])

        shifted = AP(t["em"].tensor, 2 * T, [[G * T, P], [1, G * T]])
        nc.sync.dma_start(t["emS"][:], shifted)
        permt = crf.tile([P, 1], I32, name="permt", tag="permt")
        nc.sync.dma_start(permt[:], t["permC"][:])
        e2 = crf.tile([P, G * T], F32, name="e2", tag="e2")
        nc.gpsimd.indirect_dma_start(
            out=e2[:], out_offset=None, in_=t["emS"][:],
            in_offset=bass.IndirectOffsetOnAxis(ap=permt[:, :1], axis=0),
        )

        # M[c, g, i, j] = ilog + mask*(trans + e - ilog)
        m0 = crf.tile([P, G, 81], F32, name="m0", tag="m0")
        mv = m0[:].rearrange("p g (i j) -> p g i j", i=T)
        e2v = e2[:].rearrange("p (g j) -> p g j", g=G)
        e2v = e2v.unsqueeze(2).broadcast_to([P, G, T, T])
        trv = transB[:].rearrange("p (i j) -> p i j", i=T)
        trv = trv.unsqueeze(1).broadcast_to([P, G, T, T])
        nc.vector.tensor_tensor(out=mv, in0=trv, in1=e2v, op=ALU.add)
        ilv = ilogB[:].rearrange("p (i j) -> p i j", i=T)
        ilv = ilv.unsqueeze(1).broadcast_to([P, G, T, T])
        nc.vector.tensor_tensor(out=mv, in0=mv, in1=ilv, op=ALU.subtract)
        mkv = maskB[:].unsqueeze(2).unsqueeze(3).broadcast_to([P, G, T, T])
        nc.vector.tensor_tensor(out=mv, in0=mv, in1=mkv, op=ALU.mult)
        nc.vector.tensor_tensor(out=mv, in0=mv, in1=ilv, op=ALU.add)

        # in-chunk combines 8 -> 4 -> 2 -> 1 in LINEAR domain (one upfront
        # exp, multiply+reduce only; path sums bounded so fp32 never
        # overflows: chunk products <= ~2e16, one cross level <= ~5e33)
        mlin = crf.tile([P, G, 81], F32, name="mlin", tag="mlin")
        nc.scalar.activation(mlin[:].rearrange("p g x -> p (g x)"),
                             m0[:].rearrange("p g x -> p (g x)"), AF.Exp)
        cur = mlin
        width = G
        lvl = 0
        while width > 1:
            width //= 2
            nxt = crf.tile([P, width, 81], F32, name=f"ml{lvl}", tag=f"ml{lvl}")
            pairs = cur[:].rearrange("p a x -> p a x")
            av = pairs[:, 0:2 * width:2, :]
            bv = pairs[:, 1:2 * width:2, :]
            _crf_combine_lin(nc, nxt[:], av, bv, crfs, P, width)
            cur = nxt
            lvl += 1

        # first cross-chunk level (128 -> 64) still linear, then to log
        cur_ap = cur[:].rearrange("p a x -> p (a x)")   # [128, 81]
        bL = crf.tile([P, 81], F32, name="tbL", tag="tbL")
        nc.sync.dma_start(bL[:64, :], cur_ap[64:128])
        nxtL = crf.tile([P, 81], F32, name="tnL", tag="tnL")
        _crf_combine_lin(nc, nxtL[:64].unsqueeze(1),
                         cur_ap[:64].unsqueeze(1), bL[:64].unsqueeze(1),
                         crfs, 64, 1)
        logc = crf.tile([P, 81], F32, name="logc", tag="logc")
        nc.scalar.activation(logc[:64, :], nxtL[:64, :], AF.Ln)

        # remaining cross-chunk tree (64 -> 2) in log domain, stabilized
        nact = 64
        cur_ap = logc[:]
        while nact > 2:
            half = nact // 2
            bT = crf.tile([P, 81], F32, name=f"tb{nact}", tag=f"tb{nact}")
            nc.sync.dma_start(bT[:half, :], cur_ap[half:nact])
            nxt = crf.tile([P, 81], F32, name=f"tn{nact}", tag=f"tn{nact}")
            _crf_combine(nc,
                         nxt[:half].unsqueeze(1),
                         cur_ap[:half].unsqueeze(1),
                         bT[:half].unsqueeze(1),
                         crfs, half, 1)
            cur_ap = nxt[:]
            nact = half

        # alpha0 = start + em[:, row 1]; alphaF = alpha0 'logvecmat' Ptot
        a0 = crf.tile([BL, T], F32, name="a0", tag="a0")
        src0 = AP(t["em"].tensor, T, [[S * T, BL], [1, T]])
        nc.sync.dma_start(a0[:], src0)
        st2 = crf.tile([BL, T], F32, name="st2", tag="st2")
        nc.sync.dma_start(st2[:], t["start2"][:])
        nc.vector.tensor_add(a0[:], a0[:], st2[:])

        s0 = crf.tile([BL, T, T], F32, name="s0", tag="s0")   # [b, j, k]
        a0v = a0[:].unsqueeze(1).broadcast_to([BL, T, T])          # k inner
        pv = cur_ap[:BL].rearrange("p (k j) -> p k j", k=T)
        pv = pv.transpose([0, 2, 1])                               # [b, j, k]
        nc.vector.tensor_tensor(out=s0[:], in0=a0v, in1=pv, op=ALU.add)
        mx0 = crf.tile([BL, T], F32, name="mx0", tag="mx0")
        nc.vector.reduce_max(out=mx0[:], in_=s0[:], axis=AX.X)
        mx0v = mx0[:].unsqueeze(2).broadcast_to([BL, T, T])
        nc.vector.tensor_tensor(out=s0[:], in0=s0[:], in1=mx0v,
                                op=ALU.subtract)
        nc.scalar.activation(s0[:], s0[:], AF.Exp)
        sm0 = crf.tile([BL, T], F32, name="sm0", tag="sm0")
        nc.vector.reduce_sum(out=sm0[:], in_=s0[:], axis=AX.X)
        nc.scalar.activation(sm0[:], sm0[:], AF.Ln)
        af = crf.tile([BL, T], F32, name="af", tag="af")
        nc.vector.tensor_add(af[:], sm0[:], mx0[:])
        en2 = crf.tile([BL, T], F32, name="en2", tag="en2")
        nc.sync.dma_start(en2[:], t["end2"][:])
        nc.vector.tensor_add(af[:], af[:], en2[:])
        mx1 = crf.tile([BL, 1], F32, name="mx1", tag="mx1")
        nc.vector.reduce_max(out=mx1[:], in_=af[:], axis=AX.X)
        nc.vector.tensor_scalar(out=af[:], in0=af[:], scalar1=mx1[:, :1],
                                scalar2=None, op0=ALU.subtract)
        nc.scalar.activation(af[:], af[:], AF.Exp)
        sm1 = crf.tile([BL, 1], F32, name="sm1", tag="sm1")
        nc.vector.reduce_sum(out=sm1[:], in_=af[:], axis=AX.X)
        nc.scalar.activation(sm1[:], sm1[:], AF.Ln)
        lz = crf.tile([BL, 1], F32, name="lz", tag="lz")
        nc.vector.tensor_add(lz[:], sm1[:], mx1[:])
        nc.sync.dma_start(t["logz"][:], lz[:])


# ----------------------------------------------------------------------------
# host side
# ----------------------------------------------------------------------------

_NC_CACHE = None
last_exec_time_ns = None
last_results = None


def _get_nc():
    global _NC_CACHE
    if _NC_CACHE is None:
        _NC_CACHE = build_program()
    return _NC_CACHE


def _pairw(w):
    """[Din, Dout] -> [Din//256, 128, 2, Dout] DoubleRow pair layout."""
    din, dout = w.shape
    return np.ascontiguousarray(
        w.reshape(din // 256, 2, P, dout).transpose(0, 2, 1, 3))


def _prep_inputs(inputs):
    """Build the 8 per-core input maps (numpy only)."""
    f8 = ml_dtypes.float8_e4m3
    bf = ml_dtypes.bfloat16
    f32 = np.float32
    x = np.asarray(inputs["x"]).astype(np.int64)
    y = np.asarray(inputs["y"]).astype(np.int64)
    g = {k: np.asarray(v).astype(f32) for k, v in inputs.items()
         if k not in ("x", "y")}

    shared = {}
    shared["wemb"] = g["word_emb"]
    shared["pos"] = g["pos_emb"]

    wqkv8 = np.empty((L, KP, P, 2, 3 * D), f8)
    qkbT = np.empty((L, 96, 16), np.float32)
    vbB = np.empty((L, 1, D), np.float32)
    for l in range(L):
        wq = g["Wqkv"][l][:, :D][:, _PQK]
        wk = g["Wqkv"][l][:, D:2 * D][:, _PQK]
        wv = g["Wqkv"][l][:, 2 * D:]
        wl = np.concatenate([wq, wk, wv], axis=1) * WS
        wqkv8[l] = _pairw(wl.astype(f8))
        bq = g["bqkv"][l][:D][_PQK]
        bk = g["bqkv"][l][D:2 * D][_PQK]
        qkbT[l] = (np.concatenate([bq, bk]) * WS).reshape(16, 96).T
        vbB[l, 0] = g["bqkv"][l][2 * D:] * WS
    shared["wqkv8"] = wqkv8
    shared["qkbT"] = qkbT
    shared["vbB"] = vbB.astype(bf)
    shared["wo8"] = np.stack([_pairw((g["Wo"][l] * WS).astype(f8))
                              for l in range(L)])
    shared["w18"] = np.stack([_pairw((g["W1"][l] * WS).astype(f8))
                              for l in range(L)])
    shared["w28"] = np.stack([_pairw((g["W2"][l] * WS).astype(f8))
                              for l in range(L)])
    wtp = np.zeros((D, 32), np.float32)
    wtp[:, :T] = g["W_tag"] * WS
    shared["wtag8"] = _pairw(wtp.astype(f8))
    shared["boB"] = (g["bo"][:, None, :] * WS).astype(bf)
    shared["b2B"] = (g["b2"][:, None, :] * WS).astype(bf)
    shared["b1T"] = np.ascontiguousarray(
        g["b1"].reshape(L, FF // P, P).transpose(0, 2, 1))
    lnsT = np.stack([g["ln_e_s"]] + [g[f"ln{i}_s"][l] for l in range(L)
                                     for i in (1, 2)])
    lnbT = np.stack([g["ln_e_b"]] + [g[f"ln{i}_b"][l] for l in range(L)
                                     for i in (1, 2)])
    shared["lnsT"] = np.ascontiguousarray(
        lnsT.reshape(2 * L + 1, KD, P).transpose(0, 2, 1))
    shared["lnbT"] = np.ascontiguousarray(
        lnbT.reshape(2 * L + 1, KD, P).transpose(0, 2, 1))
    shared["btag"] = g["b_tag"].reshape(T, 1).copy()
    trans = g["crf_trans"]
    shared["transB"] = np.broadcast_to(trans.reshape(1, 81), (P, 81)).copy()
    ilog = np.full((T, T), NEG, f32)
    np.fill_diagonal(ilog, 0.0)
    shared["ilogB"] = np.broadcast_to(ilog.reshape(1, 81), (P, 81)).copy()
    shared["start2"] = np.broadcast_to(g["crf_start"], (BL, T)).copy()
    shared["permC"] = _BITREV7.reshape(P, 1).astype(np.int32)
    shared["end2"] = np.broadcast_to(g["crf_end"], (BL, T)).copy()

    in_maps = []
    num_consts = []
    for c in range(NCORES):
        xs = x[c * BL:(c + 1) * BL]           # [BL, S]
        ys = y[c * BL:(c + 1) * BL]
        m = {}
        m.update(shared)
        m["tok"] = np.ascontiguousarray(
            xs.reshape(NTOK, 1).astype(np.int32))

        tags = ys[:, 1:]                       # [BL, 511]
        mask = (tags > 0)
        mf = mask.astype(f32)
        mrow = np.zeros((BL, CCH * G), f32)
        mrow[:, :NSTEP] = mf[:, 1:]
        m["maskB"] = np.ascontiguousarray(
            mrow.reshape(BL * CCH, G)[_BITREV7])
        sel = np.zeros((BL, S, T), f32)
        bi = np.arange(BL)[:, None]
        tpos = np.arange(S - 1)[None, :]
        w = np.concatenate([np.ones((BL, 1), f32), mf[:, 1:]], axis=1)
        sel[bi, tpos + 1, tags] = w
        m["selT"] = np.ascontiguousarray(sel.reshape(NTOK, T).T)
        in_maps.append(m)

        tr = trans[tags[:, :-1], tags[:, 1:]]
        num_c = g["crf_start"][tags[:, 0]].sum()
        num_c += (tr * mf[:, 1:]).sum()
        last = mask.sum(axis=1).astype(np.int64) - 1
        num_c += g["crf_end"][tags[np.arange(BL), last]].sum()
        num_consts.append(float(num_c))
    return in_maps, num_consts


def kernel(**inputs):
    global last_exec_time_ns
    import os
    nc = _get_nc()
    in_maps, num_consts = _prep_inputs(inputs)
    trace = bool(int(os.environ.get("KERNEL_TRACE", "0")))
    if trace:
        import concourse.bass_utils as _BU
        _BU.upload_artifacts = lambda tmpdir: tmpdir
        tdir = os.environ.get("KERNEL_TRACE_DIR")
        if tdir:
            os.makedirs(tdir, exist_ok=True)
        try:
            res = run_bass_kernel_spmd(
                nc, in_maps, core_ids=list(range(NCORES)), trace=True,
                tmpdir=tdir)
            global last_results
            last_results = res
        except Exception as e:
            print(f"trace run failed ({e!r}); retrying untraced")
            res = run_bass_kernel_spmd(
                nc, in_maps, core_ids=list(range(NCORES)), trace=False)
    else:
        res = run_bass_kernel_spmd(
            nc, in_maps, core_ids=list(range(NCORES)), trace=False)
    last_exec_time_ns = res.exec_time_ns
    loss = 0.0
    for c in range(NCORES):
        r = res.results[c]
        num = num_consts[c] + float(r["numdot"].sum())
        logz = float(r["logz"].sum())
        loss += logz - num
    return np.float32(loss)


# revision 45
# speedup vs baseline: 1.3212x; 1.0015x over previous
"""BERT(2-layer) + CRF NLL loss kernel for Trainium2, data-parallel over batch on 8 cores.

fp8 (e4m3) DoubleRow matmuls for all linear layers (halves PSUM accumulation
passes: K=256 per instruction), bf16 pre-LN accumulators, fp8 residual stream
(scale 1, weights x64), DVE-only rsqrt for LN (bit-trick seed + 1 Newton step,
768-scale folded into the Newton constants -- no Ln/Exp act-table loads on the
LN critical path), QK bias folded into the PSUM-drain tensor_scalar.

Layout per core (2 examples, 1024 token-slots), all feature-major:
  - h8   fp8 [128, 2, 1024] x3: post-LN activations, pair i = feature k-tile 2p+i.
  - q8/k8 fp8 [96, 2, 1024] x4: tile a holds heads 3a..3a+2 at partition bases
    0/32/64 (PE cannot read base 96), pair i = dh 32i+r, via a host-side
    wqkv column permutation; scores contract DH=64 as [32,2] DoubleRow.
  - v8 fp8 [128, 2, 768] x4 token-groups (token (2u+i)*128+p); softmax
    denominator via a separate ones[128,2,32] DoubleRow matmul (DoubleRow
    stationary must be >=32 columns).
  - ctx8 fp8 x3 natural feature pairs; xres bf16 x6: pre-LN residual accumulators.
  - weights fp8 x64 in DoubleRow pair layout [128, 2, out]; Wo/W2/V biases via
    small bf16 rank-1 matmuls (64*b rows x ones) accumulated into PSUM; W1
    bias via the gelu activation bias; QK bias via per-partition tensor_scalar.
  - FF runs in 256-token chunks so W1/gelu/W2 pipeline inside 8 PSUM banks.
  - CRF identical to v1 (log-domain associative scan over 9x9 matrices).

Known hw caveats found on the way: DoubleRow is 1 cycle/row on TRN2 (cost
model says 0.5) -- the win is 2x K per instruction, not faster rows; the
chip power-throttles the PE to ~50% duty for ~2/3 of this kernel (dense
PE-only streams do not trip it); tensor_tensor_reduce crashes the device;
engine ops need partition bases in {0,32,64,96}.
"""

import sys

sys.path.insert(0, "/opt/trn_rl_repo")

import numpy as np
import ml_dtypes

import concourse.bass as bass
import concourse.tile as tile
from concourse import bacc, mybir
from concourse.bass import AP
from concourse.bass_utils import run_bass_kernel_spmd
from concourse.masks import make_identity

F32 = mybir.dt.float32
BF16 = mybir.dt.bfloat16
FP8 = mybir.dt.float8e4
I32 = mybir.dt.int32
AF = mybir.ActivationFunctionType
ALU = mybir.AluOpType
AX = mybir.AxisListType
DR = mybir.MatmulPerfMode.DoubleRow

P = 128
B, S, D, L, H, T, V = 16, 512, 768, 2, 12, 9, 30522
DH = D // H          # 64
FF = 4 * D           # 3072
NCORES = 8
BL = B // NCORES     # 2 examples per core
NTOK = BL * S        # 1024
KD = D // P          # 6 k-tiles over D
KP = KD // 2         # 3 k-pair tiles
FFP = FF // 256      # 12 ff-pair tiles
NT = NTOK // 512     # 2 n-chunks of 512 tokens
TT = NTOK // P       # 8 token-tiles
EPS = 1e-12
NEG = -1000.0        # effective -inf for log-domain CRF
G = 8                # CRF scan steps per chunk
CCH = 64             # chunks per example
NSTEP = 510          # scan steps (S'-1 where S'=511)
EMROWS = NTOK + 16   # em output padded so chunk loads never go OOB
WS = 64.0            # fp8 weight scale
IWS = 1.0 / WS
ES = 16.0            # exp tile scale
LN16 = float(np.log(ES))
LN768 = float(np.log(768.0))
EPS_S = 768.0 * 768.0 * EPS

def _bitrev(n, bits):
    r = 0
    for _ in range(bits):
        r = (r << 1) | (n & 1)
        n >>= 1
    return r

_BITREV7 = np.array([_bitrev(p, 7) for p in range(128)], dtype=np.int64)

# q/k output-feature permutation: tile a holds heads 3a..3a+2 on partitions
# 32j+r (j=head%3, base 0/32/64 only -- PE cannot read from base 96), pair i
# selects dh 32i+r. Permuted column a*192 + i*96 + j*32 + r <- head 3a+j,
# dh i*32+r.
_PQK = np.empty(D, dtype=np.int64)
for _a in range(4):
    for _i in range(2):
        for _j in range(3):
            for _r in range(32):
                _PQK[_a * 192 + _i * 96 + _j * 32 + _r] = \
                    (3 * _a + _j) * 64 + _i * 32 + _r


# ----------------------------------------------------------------------------
# device program
# ----------------------------------------------------------------------------

def build_program():
    nc = bacc.Bacc("TRN2", target_bir_lowering=False, debug=False)

    def din(name, shape, dt):
        return nc.dram_tensor(name, shape, dt, kind="ExternalInput").ap()

    def dout(name, shape, dt):
        return nc.dram_tensor(name, shape, dt, kind="ExternalOutput").ap()

    t = dict(
        tok=din("tok", [NTOK, 1], I32),
        wemb=din("wemb", [V, D], F32),
        pos=din("pos", [S, D], F32),
        wqkv8=din("wqkv8", [L, KP, P, 2, 3 * D], FP8),
        wo8=din("wo8", [L, KP, P, 2, D], FP8),
        w18=din("w18", [L, KP, P, 2, FF], FP8),
        w28=din("w28", [L, FFP, P, 2, D], FP8),
        wtag8=din("wtag8", [KP, P, 2, 32], FP8),
        qkbT=din("qkbT", [L, 96, 16], F32),      # 64*b, permuted, per-tile cols
        vbB=din("vbB", [L, 1, D], BF16),         # 64*b_v
        boB=din("boB", [L, 1, D], BF16),
        b2B=din("b2B", [L, 1, D], BF16),
        b1T=din("b1T", [L, P, FF // P], F32),
        lnsT=din("lnsT", [2 * L + 1, P, KD], F32),
        lnbT=din("lnbT", [2 * L + 1, P, KD], F32),
        btag=din("btag", [T, 1], F32),
        transB=din("transB", [P, 81], F32),
        ilogB=din("ilogB", [P, 81], F32),
        maskB=din("maskB", [P, G], F32),
        start2=din("start2", [BL, T], F32),
        end2=din("end2", [BL, T], F32),
        selT=din("selT", [T, NTOK], F32),
        permC=din("permC", [P, 1], I32),
        emS=nc.dram_tensor("emS", [P, G * T], F32, kind="Internal").ap(),
        em=dout("em", [EMROWS, T], F32),
        numdot=dout("numdot", [T, 1], F32),
        logz=dout("logz", [BL, 1], F32),
    )

    with tile.TileContext(nc) as tc:
        _emit(nc, tc, t)
    nc.compile()
    return nc


def _emit(nc, tc, t):
    from contextlib import ExitStack

    with ExitStack() as ctx:
        const = ctx.enter_context(tc.tile_pool(name="const", bufs=1))
        hpool = ctx.enter_context(tc.tile_pool(name="h", bufs=1))
        wpool = ctx.enter_context(tc.tile_pool(name="w", bufs=1))

        ident = const.tile([P, P], F32, name="ident", tag="ident")
        make_identity(nc, ident[:])
        identb = const.tile([P, P], BF16, name="identb", tag="identb")
        make_identity(nc, identb[:])
        ones_row = const.tile([1, 512], BF16, name="ones_row", tag="ones_row")
        nc.vector.memset(ones_row[:], 1.0)
        ones1b = const.tile([1, P], BF16, name="ones1b", tag="ones1b")
        nc.vector.memset(ones1b[:], 1.0)
        c768row = const.tile([1, P], BF16, name="c768row", tag="c768row")
        nc.vector.memset(c768row[:], 1.0 / 768.0)
        ones128b = const.tile([P, 1], BF16, name="ones128b", tag="ones128b")
        nc.vector.memset(ones128b[:], 1.0)
        epsS = const.tile([1, 1], F32, name="epsS", tag="epsS")
        nc.vector.memset(epsS[:], EPS_S)
        ln768t = const.tile([1, 1], F32, name="ln768t", tag="ln768t")
        nc.vector.memset(ln768t[:], LN768)
        ln16t = const.tile([P, 1], F32, name="ln16t", tag="ln16t")
        nc.vector.memset(ln16t[:], LN16)
        cbias = dict(epsS=epsS, ln768t=ln768t, ln16t=ln16t)  # + ones8 below

        # persistent activation tiles
        h8 = [hpool.tile([P, 2, NTOK], FP8, name=f"h8_{p}", tag=f"h8_{p}")
              for p in range(KP)]
        q8 = [hpool.tile([96, 2, NTOK], FP8, name=f"q8_{a}", tag=f"q8_{a}")
              for a in range(4)]
        k8 = [hpool.tile([96, 2, NTOK], FP8, name=f"k8_{a}", tag=f"k8_{a}")
              for a in range(4)]
        v8 = [hpool.tile([P, 2, H * DH], FP8, name=f"v8_{u}", tag=f"v8_{u}")
              for u in range(4)]
        ones8 = const.tile([P, 2, 32], FP8, name="ones8", tag="ones8")
        nc.vector.memset(ones8[:], 1.0)
        cbias["ones8"] = ones8
        ctx8 = [hpool.tile([P, 2, NTOK], FP8, name=f"ctx8_{p}", tag=f"ctx8_{p}")
                for p in range(KP)]
        xres = [hpool.tile([P, NTOK], BF16, name=f"xres{k}", tag=f"xres{k}")
                for k in range(KD)]

        # LN scale/bias param tiles ([P, site, k])
        lns = const.tile([P, 2 * L + 1, KD], F32, name="lns", tag="lns")
        nc.sync.dma_start(lns[:], t["lnsT"].rearrange("a p k -> p a k"))
        lnb = const.tile([P, 2 * L + 1, KD], F32, name="lnb", tag="lnb")
        nc.sync.dma_start(lnb[:], t["lnbT"].rearrange("a p k -> p a k"))

        # ------------------------------------------------------------------
        # embedding: gather + pos, cast bf16, transpose to xres, then LN
        # ------------------------------------------------------------------
        with tc.tile_pool(name="emb", bufs=3) as emb, \
             tc.tile_pool(name="embps", bufs=3, space="PSUM") as embps, \
             tc.tile_pool(name="posp", bufs=1) as posp:
            pos_t = []
            for q in range(S // P):
                pt = posp.tile([P, D], F32, name=f"pos{q}", tag=f"pos{q}")
                nc.sync.dma_start(pt[:], t["pos"][q * P:(q + 1) * P, :])
                pos_t.append(pt)
            for n in range(NT):
                gbfs = []
                for q in range(4):
                    tt_i = n * 4 + q
                    idx = emb.tile([P, 1], I32, name="idx", tag="idx")
                    nc.sync.dma_start(idx[:], t["tok"][tt_i * P:(tt_i + 1) * P, :])
                    g32 = emb.tile([P, D], F32, name="g32", tag="g32")
                    nc.gpsimd.indirect_dma_start(
                        out=g32[:], out_offset=None, in_=t["wemb"][:],
                        in_offset=bass.IndirectOffsetOnAxis(ap=idx[:, :1], axis=0),
                    )
                    gbf = emb.tile([P, D], BF16, name=f"gbf{q}", tag=f"gbf{q}")
                    nc.vector.tensor_add(gbf[:], g32[:], pos_t[tt_i % 4][:])
                    gbfs.append(gbf)
                for k in range(KD):
                    pb = embps.tile([P, 4, P], BF16, name="pb", tag="pb",
                                    space="PSUM")
                    for q in range(4):
                        nc.tensor.transpose(
                            pb[:, q, :], gbfs[q][:, k * P:(k + 1) * P], identb[:])
                    nc.vector.tensor_copy(
                        xres[k][:, n * 512:(n + 1) * 512],
                        pb[:].rearrange("p q c -> p (q c)"))
        for n in range(NT):
            _ln_apply(nc, tc, n, xres, h8, lns[:, 0, :], lnb[:, 0, :],
                      ones128b, ones1b, c768row, cbias)

        # ------------------------------------------------------------------
        # weights to SBUF (fp8)
        # ------------------------------------------------------------------
        wq8 = [[wpool.tile([P, 2, 3 * D], FP8, name=f"wq{l}_{p}", tag=f"wq{l}_{p}")
                for p in range(KP)] for l in range(L)]
        wo8 = [[wpool.tile([P, 2, D], FP8, name=f"wo{l}_{p}", tag=f"wo{l}_{p}")
                for p in range(KP)] for l in range(L)]
        w18 = [[wpool.tile([P, 2, FF], FP8, name=f"w1{l}_{p}", tag=f"w1{l}_{p}")
                for p in range(KP)] for l in range(L)]
        w28 = [[wpool.tile([P, 2, D], FP8, name=f"w2{l}_{f}", tag=f"w2{l}_{f}")
                for f in range(FFP)] for l in range(L)]
        for l in range(L):
            for p in range(KP):
                nc.sync.dma_start(wq8[l][p][:], t["wqkv8"][l, p])
                nc.sync.dma_start(wo8[l][p][:], t["wo8"][l, p])
                nc.sync.dma_start(w18[l][p][:], t["w18"][l, p])
            for f in range(FFP):
                nc.sync.dma_start(w28[l][f][:], t["w28"][l, f])
        qkb = const.tile([96, L, 16], F32, name="qkb", tag="qkb")
        nc.sync.dma_start(qkb[:], t["qkbT"].rearrange("l p m -> p l m"))
        vbb = const.tile([1, L, D], BF16, name="vbb", tag="vbb")
        nc.sync.dma_start(vbb[:], t["vbB"].rearrange("l o d -> o l d"))
        bob = const.tile([1, L, D], BF16, name="bob", tag="bob")
        nc.sync.dma_start(bob[:], t["boB"].rearrange("l o d -> o l d"))
        b2b = const.tile([1, L, D], BF16, name="b2b", tag="b2b")
        nc.sync.dma_start(b2b[:], t["b2B"].rearrange("l o d -> o l d"))
        b1t = const.tile([P, L, FF // P], F32, name="b1t", tag="b1t")
        nc.sync.dma_start(b1t[:], t["b1T"].rearrange("l p k -> p l k"))

        # ------------------------------------------------------------------
        # encoder layers
        # ------------------------------------------------------------------
        for l in range(L):
            _layer(nc, tc, t, l, h8, q8, k8, v8, ctx8, xres,
                   wq8[l], wo8[l], w18[l], w28[l],
                   qkb, vbb[:, l, :], bob[:, l, :], b2b[:, l, :],
                   b1t[:, l, :],
                   lns, lnb, ones_row, ones1b, c768row, ones128b, cbias)

        # ------------------------------------------------------------------
        # emissions: em = wtag.T @ h + btag  (feature-major [9, NTOK])
        # ------------------------------------------------------------------
        with tc.tile_pool(name="emp", bufs=1) as emp, \
             tc.tile_pool(name="emps", bufs=2, space="PSUM") as emps:
            wtg = [emp.tile([P, 2, 32], FP8, name=f"wtg{p}", tag=f"wtg{p}")
                   for p in range(KP)]
            for p in range(KP):
                nc.sync.dma_start(wtg[p][:], t["wtag8"][p])
            btg = emp.tile([T, 1], F32, name="btg", tag="btg")
            nc.sync.dma_start(btg[:], t["btag"][:])
            em_sb = emp.tile([T, NTOK], F32, name="em_sb", tag="em_sb")
            for n in range(NT):
                ps = emps.tile([32, 512], F32, name="emmm", tag="emmm",
                               space="PSUM")
                for p in range(KP):
                    nc.tensor.matmul(
                        ps[:], lhsT=wtg[p][:],
                        rhs=h8[p][:, :, n * 512:(n + 1) * 512],
                        start=(p == 0), stop=(p == KP - 1), perf_mode=DR)
                nc.scalar.activation(
                    em_sb[:, n * 512:(n + 1) * 512], ps[:T, :], AF.Identity,
                    bias=btg[:, :1], scale=IWS)
            # numerator dot: sum(em * selT) fused multiply-reduce
            sel = emp.tile([T, NTOK], F32, name="sel", tag="sel")
            nc.sync.dma_start(sel[:], t["selT"][:])
            prod = emp.tile([T, NTOK], F32, name="prod", tag="prod")
            nc.vector.tensor_mul(prod[:], em_sb[:], sel[:])
            nd = emp.tile([T, 1], F32, name="nd", tag="nd")
            nc.vector.reduce_sum(out=nd[:], in_=prod[:], axis=AX.X)
            nc.sync.dma_start(t["numdot"][:], nd[:])
            # token-major em to DRAM (+ zero pad rows)
            zpad = emp.tile([16, T], F32, name="zpad", tag="zpad")
            nc.vector.memset(zpad[:], 0.0)
            nc.sync.dma_start(t["em"][NTOK:EMROWS, :], zpad[:])
            for tt_i in range(TT):
                tp = emps.tile([P, T], F32, name="emtp", tag="emtp", space="PSUM")
                nc.tensor.transpose(
                    tp[:], em_sb[:, tt_i * P:(tt_i + 1) * P], ident[:T, :T])
                emtm = emp.tile([P, T], F32, name="emtm", tag="emtm", bufs=3)
                nc.vector.tensor_copy(emtm[:], tp[:])
                nc.sync.dma_start(t["em"][tt_i * P:(tt_i + 1) * P, :], emtm[:])

        # ------------------------------------------------------------------
        # CRF forward pass (log-domain associative scan)
        # ------------------------------------------------------------------
        _crf(nc, tc, t)


def _ln_apply(nc, tc, n, xres, h8, sT, bT, ones128b, ones1b, c768row, cbias):
    """Feature-major LN of xres (bf16) chunk n -> h8 (fp8).

    rstd computed as exp(-0.5 ln(768*sq - mu^2) + ln 768).
    """
    sl = slice(n * 512, (n + 1) * 512)
    with tc.tile_pool(name="lnp", bufs=1) as lnp, \
         tc.tile_pool(name="lnps", bufs=1, space="PSUM") as lnps:
        mu_ps = lnps.tile([1, 512], F32, name="mu", tag="mu", space="PSUM")
        sq_ps = lnps.tile([1, 512], F32, name="sq", tag="sq", space="PSUM")
        xsq = [lnp.tile([P, 512], BF16, name=f"xsq{k}", tag=f"xsq{k}", bufs=1)
               for k in range(KD)]
        for k in range(KD):
            nc.vector.tensor_mul(xsq[k][:], xres[k][:, sl], xres[k][:, sl])
        for k in range(KD):
            nc.tensor.matmul(mu_ps[:], lhsT=ones128b[:], rhs=xres[k][:, sl],
                             start=(k == 0), stop=(k == KD - 1))
        for k in range(KD):
            nc.tensor.matmul(sq_ps[:], lhsT=ones128b[:], rhs=xsq[k][:],
                             start=(k == 0), stop=(k == KD - 1))
        musq = lnp.tile([1, 512], F32, name="musq", tag="musq")
        nc.scalar.square(musq[:], mu_ps[:])
        svar = lnp.tile([1, 512], F32, name="svar", tag="svar")
        nc.vector.scalar_tensor_tensor(
            out=svar[:], in0=sq_ps[:], scalar=768.0, in1=musq[:],
            op0=ALU.mult, op1=ALU.subtract)
        # rstd = 768*rsqrt(svar) via bit-trick seed + one Newton step, all
        # on DVE (no Ln/Exp -> no act-table loads on the LN critical path)
        q1 = lnp.tile([1, 512], I32, name="q1", tag="q1")
        nc.vector.tensor_scalar(
            out=q1[:], in0=svar[:].bitcast(I32), scalar1=1, scalar2=None,
            op0=ALU.logical_shift_right)
        q2 = lnp.tile([1, 512], I32, name="q2", tag="q2")
        nc.vector.tensor_scalar(
            out=q2[:], in0=q1[:], scalar1=-1, scalar2=None,
            op0=ALU.bitwise_xor)
        y0 = lnp.tile([1, 512], F32, name="y0", tag="y0")
        nc.vector.tensor_scalar(
            out=y0[:].bitcast(I32), in0=q2[:], scalar1=0x5F3759E0,
            scalar2=None, op0=ALU.add)
        n1 = lnp.tile([1, 512], F32, name="n1", tag="n1")
        nc.vector.tensor_mul(n1[:], y0[:], y0[:])
        n2 = lnp.tile([1, 512], F32, name="n2", tag="n2")
        nc.vector.tensor_mul(n2[:], n1[:], svar[:])
        n3 = lnp.tile([1, 512], F32, name="n3", tag="n3")
        nc.vector.tensor_scalar(
            out=n3[:], in0=n2[:], scalar1=-384.0, scalar2=1152.0,
            op0=ALU.mult, op1=ALU.add)
        rs = lnp.tile([1, 512], BF16, name="rs", tag="rs")
        nc.vector.tensor_mul(rs[:], y0[:], n3[:])
        murs = lnp.tile([1, 512], BF16, name="murs", tag="murs")
        nc.vector.tensor_mul(murs[:], mu_ps[:], rs[:])
        rsB_ps = lnps.tile([P, 512], F32, name="rsB", tag="rsB", space="PSUM")
        nc.tensor.matmul(rsB_ps[:], lhsT=ones1b[:], rhs=rs[:],
                         start=True, stop=True)
        m2_ps = lnps.tile([P, 512], F32, name="m2B", tag="m2B", space="PSUM")
        nc.tensor.matmul(m2_ps[:], lhsT=c768row[:], rhs=murs[:],
                         start=True, stop=True)
        rsB = lnp.tile([P, 512], BF16, name="rsBs", tag="rsBs")
        nc.vector.tensor_copy(rsB[:], rsB_ps[:])
        m2B = lnp.tile([P, 512], BF16, name="m2Bs", tag="m2Bs")
        nc.vector.tensor_copy(m2B[:], m2_ps[:])
        for k in range(KD):
            tm = lnp.tile([P, 512], BF16, name="tm", tag="tm", bufs=3)
            nc.vector.tensor_mul(tm[:], xres[k][:, sl], rsB[:])
            ts = lnp.tile([P, 512], BF16, name="ts", tag="ts", bufs=3)
            nc.vector.tensor_sub(ts[:], tm[:], m2B[:])
            dst = h8[k // 2][:, k % 2, sl]
            nc.scalar.activation(dst, ts[:], AF.Identity,
                                 bias=bT[:, k:k + 1], scale=sT[:, k:k + 1])


def _layer(nc, tc, t, l, h8, q8, k8, v8, ctx8, xres,
           wq8, wo8, w18, w28, qkb, vbb, bob, b2b, b1t,
           lns, lnb, ones_row, ones1b, c768row, ones128b, cbias):
    # --------------- QKV: q8/k8 feature-major perm, v8 token-major ---------
    with tc.tile_pool(name="qkps", bufs=4, space="PSUM") as qkps:
        for n in range(NT):
            nsl = slice(n * 512, (n + 1) * 512)
            for m in range(16):       # 8 Q tiles then 8 K tiles (96-part, perm)
                csl = slice(m * 96, (m + 1) * 96)
                ps = qkps.tile([96, 512], F32, name="ps", tag="ps", space="PSUM")
                for p in range(KP):
                    nc.tensor.matmul(
                        ps[:], lhsT=wq8[p][:, :, csl],
                        rhs=h8[p][:, :, nsl],
                        start=(p == 0), stop=(p == KP - 1), perf_mode=DR,
                        skip_group_check=True)
                dstq = q8 if m < 8 else k8
                mm = m % 8
                nc.vector.tensor_scalar(
                    out=dstq[mm // 2][:, mm % 2, nsl], in0=ps[:],
                    scalar1=qkb[:, l, m:m + 1], scalar2=IWS,
                    op0=ALU.add, op1=ALU.mult)
        for tt_i in range(TT):        # V token-major
            for nv in range(2):
                vsl = slice(2 * D + nv * 384, 2 * D + (nv + 1) * 384)
                ps = qkps.tile([P, 384], F32, name="psv", tag="psv", space="PSUM")
                nc.tensor.matmul(ps[:], lhsT=ones1b[:],
                                 rhs=vbb[:, nv * 384:(nv + 1) * 384],
                                 start=True, stop=False, skip_group_check=True)
                for p in range(KP):
                    nc.tensor.matmul(
                        ps[:], lhsT=h8[p][:, :, tt_i * P:(tt_i + 1) * P],
                        rhs=wq8[p][:, :, vsl],
                        start=False, stop=(p == KP - 1), perf_mode=DR,
                        skip_group_check=True)
                nc.vector.tensor_scalar_mul(
                    v8[tt_i // 2][:, tt_i % 2, nv * 384:(nv + 1) * 384],
                    ps[:], IWS)

    # --------------- attention ----------------------------------------
    with tc.tile_pool(name="att", bufs=1) as att, \
         tc.tile_pool(name="scps", bufs=2, space="PSUM") as scps, \
         tc.tile_pool(name="ctxps", bufs=2, space="PSUM") as ctxps, \
         tc.tile_pool(name="denps", bufs=1, space="PSUM") as denps, \
         tc.tile_pool(name="invps", bufs=1, space="PSUM") as invps:
        for b in range(BL):
            bsl = slice(b * S, (b + 1) * S)
            for hp in range(H // 2):
                cps = []
                ivs = []
                for hh in range(2):
                    h = hp * 2 + hh
                    a, j = h // 3, h % 3
                    psl = slice(32 * j, 32 * j + 32)
                    cp = ctxps.tile([DH, 512], F32, name="ctx", tag="ctx",
                                    space="PSUM")
                    den = denps.tile([32, 512], F32, name="den", tag="den",
                                     space="PSUM")
                    for u in range(2):
                        sc = scps.tile([P, 2, 512], F32, name="sc", tag="sc",
                                       space="PSUM")
                        for i in range(2):
                            kt = 2 * u + i
                            nc.tensor.matmul(
                                sc[:, i, :],
                                lhsT=k8[a][psl, :,
                                           b * S + kt * P:b * S + (kt + 1) * P],
                                rhs=q8[a][psl, :, bsl],
                                start=True, stop=True, perf_mode=DR)
                        et = att.tile([P, 2, 512], FP8, name="et", tag="et",
                                      bufs=6)
                        nc.scalar.activation(
                            et[:].rearrange("p i q -> p (i q)"),
                            sc[:].rearrange("p i q -> p (i q)"),
                            AF.Exp, scale=0.125, bias=cbias["ln16t"][:, :1])
                        nc.tensor.matmul(
                            cp[:],
                            lhsT=v8[2 * b + u][:, :, h * DH:(h + 1) * DH],
                            rhs=et[:], start=(u == 0), stop=(u == 1),
                            perf_mode=DR, skip_group_check=True)
                        nc.tensor.matmul(
                            den[:], lhsT=cbias["ones8"][:], rhs=et[:],
                            start=(u == 0), stop=(u == 1),
                            perf_mode=DR, skip_group_check=True)
                    iv = att.tile([1, 512], F32, name="iv", tag="iv", bufs=6)
                    nc.vector.reciprocal_approx_fast(iv[:], den[:1, :])
                    ivb = att.tile([1, 512], BF16, name="ivb", tag="ivb",
                                   bufs=6)
                    nc.vector.tensor_copy(ivb[:], iv[:])
                    cps.append(cp)
                    ivs.append(ivb)
                ivB = invps.tile([P, 512], F32, name="ivB", tag="ivB",
                                 space="PSUM")
                nc.tensor.matmul(ivB[:DH, :], lhsT=ones1b[:, :DH],
                                 rhs=ivs[0][:], start=True, stop=True)
                nc.tensor.matmul(ivB[DH:, :], lhsT=ones1b[:, :DH],
                                 rhs=ivs[1][:], start=True, stop=True)
                ivS = att.tile([P, 512], BF16, name="ivS", tag="ivS", bufs=3)
                nc.scalar.copy(ivS[:], ivB[:])
                for hh in range(2):
                    h = hp * 2 + hh
                    po = (h % 2) * DH
                    nc.vector.tensor_mul(
                        ctx8[h // 4][po:po + DH, (h // 2) % 2, bsl],
                        cps[hh][:, :], ivS[po:po + DH, :])

    # --------------- Wo + residual + LN1 -------------------------------
    with tc.tile_pool(name="wops", bufs=4, space="PSUM") as wops:
        for n in range(NT):
            nsl = slice(n * 512, (n + 1) * 512)
            for m in range(KD):
                ps = wops.tile([P, 512], F32, name="ps", tag="ps", space="PSUM")
                nc.tensor.matmul(ps[:], lhsT=bob[:, m * P:(m + 1) * P],
                                 rhs=ones_row[:], start=True, stop=False,
                                 skip_group_check=True)
                for p in range(KP):
                    nc.tensor.matmul(
                        ps[:], lhsT=wo8[p][:, :, m * P:(m + 1) * P],
                        rhs=ctx8[p][:, :, nsl],
                        start=False, stop=(p == KP - 1), perf_mode=DR,
                        skip_group_check=True)
                nc.vector.scalar_tensor_tensor(
                    out=xres[m][:, nsl], in0=ps[:], scalar=IWS,
                    in1=h8[m // 2][:, m % 2, nsl], op0=ALU.mult, op1=ALU.add)
            _ln_apply(nc, tc, n, xres, h8, lns[:, 2 * l + 1, :],
                      lnb[:, 2 * l + 1, :], ones128b, ones1b, c768row, cbias)

    # --------------- FF (256-token chunks, pipelined in PSUM) -----------
    with tc.tile_pool(name="ffac", bufs=1, space="PSUM") as ffac, \
         tc.tile_pool(name="ffps", bufs=2, space="PSUM") as ffps, \
         tc.tile_pool(name="ffg", bufs=4) as ffg:
        acc = [ffac.tile([P, 2, 256], F32, name=f"acc{m}", tag=f"acc{m}",
                         space="PSUM") for m in range(KD)]
        for c in range(4):
            csl = slice(c * 256, (c + 1) * 256)
            ci = c % 2
            for m in range(KD):
                nc.tensor.matmul(acc[m][:, ci, :],
                                 lhsT=b2b[:, m * P:(m + 1) * P],
                                 rhs=ones_row[:, :256], start=True, stop=False,
                                 skip_group_check=True)
            for f in range(FFP):
                psg = ffps.tile([P, 2, 256], F32, name="psg", tag="psg",
                                space="PSUM")
                for i in range(2):
                    for p in range(KP):
                        nc.tensor.matmul(
                            psg[:, i, :],
                            lhsT=w18[p][:, :, (2 * f + i) * P:(2 * f + i + 1) * P],
                            rhs=h8[p][:, :, csl],
                            start=(p == 0), stop=(p == KP - 1), perf_mode=DR)
                gl = ffg.tile([P, 2, 256], FP8, name="gl", tag="gl")
                for i in range(2):
                    nc.scalar.activation(
                        gl[:, i, :], psg[:, i, :], AF.Gelu,
                        bias=b1t[:, 2 * f + i:2 * f + i + 1], scale=IWS)
                for m in range(KD):
                    nc.tensor.matmul(
                        acc[m][:, ci, :], lhsT=w28[f][:, :, m * P:(m + 1) * P],
                        rhs=gl[:], start=False, stop=(f == FFP - 1),
                        perf_mode=DR, skip_group_check=True)
            for m in range(KD):
                nc.vector.scalar_tensor_tensor(
                    out=xres[m][:, csl], in0=acc[m][:, ci, :], scalar=IWS,
                    in1=h8[m // 2][:, m % 2, csl], op0=ALU.mult, op1=ALU.add)
    for n in range(NT):
        _ln_apply(nc, tc, n, xres, h8, lns[:, 2 * l + 2, :],
                  lnb[:, 2 * l + 2, :], ones128b, ones1b, c768row, cbias)


def _crf_combine_lin(nc, out_ap, a_ap, b_ap, spool, npart, npair):
    """Linear-domain combine: out[i,j] = sum_k A[i,k]*B[k,j] (no Act ops)."""
    s = spool.tile([P, 4, 729], F32, name="cS", tag="cS")
    sv4 = s[:npart, :npair, :].rearrange("p q (x k) -> p q x k", k=T)
    for q in range(npair):
        avq = a_ap[:, q].rearrange("p (i k) -> p i k", i=T)
        avq = avq.unsqueeze(2).broadcast_to([npart, T, T, T])    # p i j k
        bvq = b_ap[:, q].rearrange("p (k j) -> p k j", k=T)
        bvq = bvq.unsqueeze(1).broadcast_to([npart, T, T, T])    # p i k j
        bvq = bvq.transpose([0, 1, 3, 2])                        # p i j k
        svq = s[:npart, q, :].rearrange("p (i j k) -> p i j k", i=T, j=T)
        nc.vector.tensor_tensor(out=svq, in0=avq, in1=bvq, op=ALU.mult)
    nc.vector.reduce_sum(out=out_ap, in_=sv4, axis=AX.X)


def _crf_combine(nc, out_ap, a_ap, b_ap, spool, npart, npair, stabilize=True):
    """out = A 'logmatmul' B over pairs: out[i,j] = lse_k(A[i,k]+B[k,j])."""
    s = spool.tile([P, 4, 729], F32, name="cS", tag="cS")
    sv4 = s[:npart, :npair, :].rearrange("p q (x k) -> p q x k", k=T)
    sv3 = s[:npart, :npair, :]
    for q in range(npair):
        avq = a_ap[:, q].rearrange("p (i k) -> p i k", i=T)
        avq = avq.unsqueeze(2).broadcast_to([npart, T, T, T])    # p i j k
        bvq = b_ap[:, q].rearrange("p (k j) -> p k j", k=T)
        bvq = bvq.unsqueeze(1).broadcast_to([npart, T, T, T])    # p i k j
        bvq = bvq.transpose([0, 1, 3, 2])                        # p i j k
        svq = s[:npart, q, :].rearrange("p (i j k) -> p i j k", i=T, j=T)
        nc.vector.tensor_tensor(out=svq, in0=avq, in1=bvq, op=ALU.add)
    sm = spool.tile([P, 4, 81], F32, name="cR", tag="cR")
    sm3 = sm[:npart, :npair, :]
    if stabilize:
        mx = spool.tile([P, 4, 81], F32, name="cM", tag="cM")
        mx3 = mx[:npart, :npair, :]
        nc.vector.reduce_max(out=mx3, in_=sv4, axis=AX.X)
        mxv = mx3.unsqueeze(3).broadcast_to([npart, npair, 81, T])
        nc.vector.tensor_tensor(out=sv4, in0=sv4, in1=mxv, op=ALU.subtract)
        nc.scalar.activation(sv3, sv3, AF.Exp)
        nc.vector.reduce_sum(out=sm3, in_=sv4, axis=AX.X)
        nc.scalar.activation(sm3, sm3, AF.Ln)
        nc.vector.tensor_tensor(out=out_ap, in0=sm3, in1=mx3, op=ALU.add)
    else:
        nc.scalar.activation(sv3, sv3, AF.Exp)
        nc.vector.reduce_sum(out=sm3, in_=sv4, axis=AX.X)
        nc.scalar.activation(out_ap, sm3, AF.Ln)


def _crf(nc, tc, t):
    """Log-domain associative scan. Partitions 0..63 = example0 chunks,
    64..127 = example1 chunks; each chunk = G=8 consecutive scan steps."""
    with tc.tile_pool(name="crf", bufs=1) as crf, \
         tc.tile_pool(name="crfs", bufs=1) as crfs:
        transB = crf.tile([P, 81], F32, name="transB", tag="transB")
        nc.sync.dma_start(transB[:], t["transB"][:])
        ilogB = crf.tile([P, 81], F32, name="ilogB", tag="ilogB")
        nc.sync.dma_start(ilogB[:], t["ilogB"][:])
        maskB = crf.tile([P, G], F32, name="maskB", tag="maskB")
        nc.sync.dma_start(maskB[:], t["maskB"][:])

        shifted = AP(t["em"].tensor, 2 * T, [[G * T, P], [1, G * T]])
        nc.sync.dma_start(t["emS"][:], shifted)
        permt = crf.tile([P, 1], I32, name="permt", tag="permt")
        nc.sync.dma_start(permt[:], t["permC"][:])
        e2 = crf.tile([P, G * T], F32, name="e2", tag="e2")
        nc.gpsimd.indirect_dma_start(
            out=e2[:], out_offset=None, in_=t["emS"][:],
            in_offset=bass.IndirectOffsetOnAxis(ap=permt[:, :1], axis=0),
        )

        # M[c, g, i, j] = ilog + mask*(trans + e - ilog)
        m0 = crf.tile([P, G, 81], F32, name="m0", tag="m0")
        mv = m0[:].rearrange("p g (i j) -> p g i j", i=T)
        e2v = e2[:].rearrange("p (g j) -> p g j", g=G)
        e2v = e2v.unsqueeze(2).broadcast_to([P, G, T, T])
        trv = transB[:].rearrange("p (i j) -> p i j", i=T)
        trv = trv.unsqueeze(1).broadcast_to([P, G, T, T])
        nc.vector.tensor_tensor(out=mv, in0=trv, in1=e2v, op=ALU.add)
        ilv = ilogB[:].rearrange("p (i j) -> p i j", i=T)
        ilv = ilv.unsqueeze(1).broadcast_to([P, G, T, T])
        nc.vector.tensor_tensor(out=mv, in0=mv, in1=ilv, op=ALU.subtract)
        mkv = maskB[:].unsqueeze(2).unsqueeze(3).broadcast_to([P, G, T, T])
        nc.vector.tensor_tensor(out=mv, in0=mv, in1=mkv, op=ALU.mult)
        nc.vector.tensor_tensor(out=mv, in0=mv, in1=ilv, op=ALU.add)

        # in-chunk combines 8 -> 4 -> 2 -> 1 in LINEAR domain (one upfront
        # exp, multiply+reduce only; path sums bounded so fp32 never
        # overflows: chunk products <= ~2e16, one cross level <= ~5e33)
        mlin = crf.tile([P, G, 81], F32, name="mlin", tag="mlin")
        nc.scalar.activation(mlin[:].rearrange("p g x -> p (g x)"),
                             m0[:].rearrange("p g x -> p (g x)"), AF.Exp)
        cur = mlin
        width = G
        lvl = 0
        while width > 1:
            width //= 2
            nxt = crf.tile([P, width, 81], F32, name=f"ml{lvl}", tag=f"ml{lvl}")
            pairs = cur[:].rearrange("p a x -> p a x")
            av = pairs[:, 0:2 * width:2, :]
            bv = pairs[:, 1:2 * width:2, :]
            _crf_combine_lin(nc, nxt[:], av, bv, crfs, P, width)
            cur = nxt
            lvl += 1

        # first cross-chunk level (128 -> 64) still linear, then to log
        cur_ap = cur[:].rearrange("p a x -> p (a x)")   # [128, 81]
        bL = crf.tile([P, 81], F32, name="tbL", tag="tbL")
        nc.sync.dma_start(bL[:64, :], cur_ap[64:128])
        nxtL = crf.tile([P, 81], F32, name="tnL", tag="tnL")
        _crf_combine_lin(nc, nxtL[:64].unsqueeze(1),
                         cur_ap[:64].unsqueeze(1), bL[:64].unsqueeze(1),
                         crfs, 64, 1)
        logc = crf.tile([P, 81], F32, name="logc", tag="logc")
        nc.scalar.activation(logc[:64, :], nxtL[:64, :], AF.Ln)

        # remaining cross-chunk tree (64 -> 2) in log domain, stabilized
        nact = 64
        cur_ap = logc[:]
        while nact > 2:
            half = nact // 2
            bT = crf.tile([P, 81], F32, name=f"tb{nact}", tag=f"tb{nact}")
            nc.sync.dma_start(bT[:half, :], cur_ap[half:nact])
            nxt = crf.tile([P, 81], F32, name=f"tn{nact}", tag=f"tn{nact}")
            _crf_combine(nc,
                         nxt[:half].unsqueeze(1),
                         cur_ap[:half].unsqueeze(1),
                         bT[:half].unsqueeze(1),
                         crfs, half, 1)
            cur_ap = nxt[:]
            nact = half

        # alpha0 = start + em[:, row 1]; alphaF = alpha0 'logvecmat' Ptot
        a0 = crf.tile([BL, T], F32, name="a0", tag="a0")
        src0 = AP(t["em"].tensor, T, [[S * T, BL], [1, T]])
        nc.sync.dma_start(a0[:], src0)
        st2 = crf.tile([BL, T], F32, name="st2", tag="st2")
        nc.sync.dma_start(st2[:], t["start2"][:])
        nc.vector.tensor_add(a0[:], a0[:], st2[:])

        s0 = crf.tile([BL, T, T], F32, name="s0", tag="s0")   # [b, j, k]
        a0v = a0[:].unsqueeze(1).broadcast_to([BL, T, T])          # k inner
        pv = cur_ap[:BL].rearrange("p (k j) -> p k j", k=T)
        pv = pv.transpose([0, 2, 1])                               # [b, j, k]
        nc.vector.tensor_tensor(out=s0[:], in0=a0v, in1=pv, op=ALU.add)
        mx0 = crf.tile([BL, T], F32, name="mx0", tag="mx0")
        nc.vector.reduce_max(out=mx0[:], in_=s0[:], axis=AX.X)
        mx0v = mx0[:].unsqueeze(2).broadcast_to([BL, T, T])
        nc.vector.tensor_tensor(out=s0[:], in0=s0[:], in1=mx0v,
                                op=ALU.subtract)
        nc.scalar.activation(s0[:], s0[:], AF.Exp)
        sm0 = crf.tile([BL, T], F32, name="sm0", tag="sm0")
        nc.vector.reduce_sum(out=sm0[:], in_=s0[:], axis=AX.X)
        nc.scalar.activation(sm0[:], sm0[:], AF.Ln)
        af = crf.tile([BL, T], F32, name="af", tag="af")
        nc.vector.tensor_add(af[:], sm0[:], mx0[:])
        en2 = crf.tile([BL, T], F32, name="en2", tag="en2")
        nc.sync.dma_start(en2[:], t["end2"][:])
        nc.vector.tensor_add(af[:], af[:], en2[:])
        mx1 = crf.tile([BL, 1], F32, name="mx1", tag="mx1")
        nc.vector.reduce_max(out=mx1[:], in_=af[:], axis=AX.X)
        nc.vector.tensor_scalar(out=af[:], in0=af[:], scalar1=mx1[:, :1],
                                scalar2=None, op0=ALU.subtract)
        nc.scalar.activation(af[:], af[:], AF.Exp)
        sm1 = crf.tile([BL, 1], F32, name="sm1", tag="sm1")
        nc.vector.reduce_sum(out=sm1[:], in_=af[:], axis=AX.X)
        nc.scalar.activation(sm1[:], sm1[:], AF.Ln)
        lz = crf.tile([BL, 1], F32, name="lz", tag="lz")
        nc.vector.tensor_add(lz[:], sm1[:], mx1[:])
        nc.sync.dma_start(t["logz"][:], lz[:])


# ----------------------------------------------------------------------------
# host side
# ----------------------------------------------------------------------------

_NC_CACHE = None
last_exec_time_ns = None
last_results = None


def _get_nc():
    global _NC_CACHE
    if _NC_CACHE is None:
        _NC_CACHE = build_program()
    return _NC_CACHE


def _pairw(w):
    """[Din, Dout] -> [Din//256, 128, 2, Dout] DoubleRow pair layout."""
    din, dout = w.shape
    return np.ascontiguousarray(
        w.reshape(din // 256, 2, P, dout).transpose(0, 2, 1, 3))


def _prep_inputs(inputs):
    """Build the 8 per-core input maps (numpy only)."""
    f8 = ml_dtypes.float8_e4m3
    bf = ml_dtypes.bfloat16
    f32 = np.float32
    x = np.asarray(inputs["x"]).astype(np.int64)
    y = np.asarray(inputs["y"]).astype(np.int64)
    g = {k: np.asarray(v).astype(f32) for k, v in inputs.items()
         if k not in ("x", "y")}

    shared = {}
    shared["wemb"] = g["word_emb"]
    shared["pos"] = g["pos_emb"]

    wqkv8 = np.empty((L, KP, P, 2, 3 * D), f8)
    qkbT = np.empty((L, 96, 16), np.float32)
    vbB = np.empty((L, 1, D), np.float32)
    for l in range(L):
        wq = g["Wqkv"][l][:, :D][:, _PQK]
        wk = g["Wqkv"][l][:, D:2 * D][:, _PQK]
        wv = g["Wqkv"][l][:, 2 * D:]
        wl = np.concatenate([wq, wk, wv], axis=1) * WS
        wqkv8[l] = _pairw(wl.astype(f8))
        bq = g["bqkv"][l][:D][_PQK]
        bk = g["bqkv"][l][D:2 * D][_PQK]
        qkbT[l] = (np.concatenate([bq, bk]) * WS).reshape(16, 96).T
        vbB[l, 0] = g["bqkv"][l][2 * D:] * WS
    shared["wqkv8"] = wqkv8
    shared["qkbT"] = qkbT
    shared["vbB"] = vbB.astype(bf)
    shared["wo8"] = np.stack([_pairw((g["Wo"][l] * WS).astype(f8))
                              for l in range(L)])
    shared["w18"] = np.stack([_pairw((g["W1"][l] * WS).astype(f8))
                              for l in range(L)])
    shared["w28"] = np.stack([_pairw((g["W2"][l] * WS).astype(f8))
                              for l in range(L)])
    wtp = np.zeros((D, 32), np.float32)
    wtp[:, :T] = g["W_tag"] * WS
    shared["wtag8"] = _pairw(wtp.astype(f8))
    shared["boB"] = (g["bo"][:, None, :] * WS).astype(bf)
    shared["b2B"] = (g["b2"][:, None, :] * WS).astype(bf)
    shared["b1T"] = np.ascontiguousarray(
        g["b1"].reshape(L, FF // P, P).transpose(0, 2, 1))
    lnsT = np.stack([g["ln_e_s"]] + [g[f"ln{i}_s"][l] for l in range(L)
                                     for i in (1, 2)])
    lnbT = np.stack([g["ln_e_b"]] + [g[f"ln{i}_b"][l] for l in range(L)
                                     for i in (1, 2)])
    shared["lnsT"] = np.ascontiguousarray(
        lnsT.reshape(2 * L + 1, KD, P).transpose(0, 2, 1))
    shared["lnbT"] = np.ascontiguousarray(
        lnbT.reshape(2 * L + 1, KD, P).transpose(0, 2, 1))
    shared["btag"] = g["b_tag"].reshape(T, 1).copy()
    trans = g["crf_trans"]
    shared["transB"] = np.broadcast_to(trans.reshape(1, 81), (P, 81)).copy()
    ilog = np.full((T, T), NEG, f32)
    np.fill_diagonal(ilog, 0.0)
    shared["ilogB"] = np.broadcast_to(ilog.reshape(1, 81), (P, 81)).copy()
    shared["start2"] = np.broadcast_to(g["crf_start"], (BL, T)).copy()
    shared["permC"] = _BITREV7.reshape(P, 1).astype(np.int32)
    shared["end2"] = np.broadcast_to(g["crf_end"], (BL, T)).copy()

    in_maps = []
    num_consts = []
    for c in range(NCORES):
        xs = x[c * BL:(c + 1) * BL]           # [BL, S]
        ys = y[c * BL:(c + 1) * BL]
        m = {}
        m.update(shared)
        m["tok"] = np.ascontiguousarray(
            xs.reshape(NTOK, 1).astype(np.int32))

        tags = ys[:, 1:]                       # [BL, 511]
        mask = (tags > 0)
        mf = mask.astype(f32)
        mrow = np.zeros((BL, CCH * G), f32)
        mrow[:, :NSTEP] = mf[:, 1:]
        m["maskB"] = np.ascontiguousarray(
            mrow.reshape(BL * CCH, G)[_BITREV7])
        sel = np.zeros((BL, S, T), f32)
        bi = np.arange(BL)[:, None]
        tpos = np.arange(S - 1)[None, :]
        w = np.concatenate([np.ones((BL, 1), f32), mf[:, 1:]], axis=1)
        sel[bi, tpos + 1, tags] = w
        m["selT"] = np.ascontiguousarray(sel.reshape(NTOK, T).T)
        in_maps.append(m)

        tr = trans[tags[:, :-1], tags[:, 1:]]
        num_c = g["crf_start"][tags[:, 0]].sum()
        num_c += (tr * mf[:, 1:]).sum()
        last = mask.sum(axis=1).astype(np.int64) - 1
        num_c += g["crf_end"][tags[np.arange(BL), last]].sum()
        num_consts.append(float(num_c))
    return in_maps, num_consts


def kernel(**inputs):
    global last_exec_time_ns
    import os
    nc = _get_nc()
    in_maps, num_consts = _prep_inputs(inputs)
    trace = bool(int(os.environ.get("KERNEL_TRACE", "0")))
    if trace:
        import concourse.bass_utils as _BU
        _BU.upload_artifacts = lambda tmpdir: tmpdir
        tdir = os.environ.get("KERNEL_TRACE_DIR")
        if tdir:
            os.makedirs(tdir, exist_ok=True)
        try:
            res = run_bass_kernel_spmd(
                nc, in_maps, core_ids=list(range(NCORES)), trace=True,
                tmpdir=tdir)
            global last_results
            last_results = res
        except Exception as e:
            print(f"trace run failed ({e!r}); retrying untraced")
            res = run_bass_kernel_spmd(
                nc, in_maps, core_ids=list(range(NCORES)), trace=False)
    else:
        res = run_bass_kernel_spmd(
            nc, in_maps, core_ids=list(range(NCORES)), trace=False)
    last_exec_time_ns = res.exec_time_ns
    loss = 0.0
    for c in range(NCORES):
        r = res.results[c]
        num = num_consts[c] + float(r["numdot"].sum())
        logz = float(r["logz"].sum())
        loss += logz - num
    return np.float32(loss)
